# revision 1
# baseline (speedup 1.0000x reference)
"""Trainium2 Bass kernel: nn_BV_Model (GENConv GNN, softmax aggregation, 4 layers).

Strategy (8 NeuronCores, SPMD):
  - Nodes are partitioned into 8 contiguous blocks (12544/core, padded).
  - Edges are sorted by destination node and bucketed per destination
    node-tile (128 nodes); each core owns the edges whose dst falls in its
    block.  Per-tile edge lists are padded to a multiple of 128 so that
    128-edge chunks never straddle node tiles (the chunk count per tile g is
    shared across cores so one SPMD program fits all cores).
  - Per layer: gather h[src] via indirect DMA with accumulate(+e) directly
    onto the preloaded edge-feature tile, compute the segment softmax
    numerator/denominator with exp (no segment-max: ranges are small enough
    for fp32, verified offline), and reduce edges->nodes with a
    one-hot(dst) matmul accumulated in PSUM.  Node MLP runs on the tensor
    engine in transposed layout.  h is AllGathered across cores per layer.
  - Global mean pool is computed per-core with a one-hot(graph) matmul,
    scattered into a [PG,C] buffer and AllReduced; every core then applies
    the output head redundantly.

Fake-quantization q(x) = clip(rne(x*1024), -32768, 32767)/1024 is computed
exactly with the round-to-nearest-even "magic number" trick (+1.5*2^23).
"""

import os
os.environ.setdefault("MYCRO_LOCAL_CACHE", "1")

import math
import numpy as np

import concourse.bacc as bacc
import concourse.tile as tile
import concourse.bass as bass
from concourse import mybir
from concourse.bass import IndirectOffsetOnAxis
from concourse.bass_utils import run_bass_kernel_spmd

F32 = mybir.dt.float32
I32 = mybir.dt.int32
ACTF = mybir.ActivationFunctionType
AL = mybir.AluOpType

MAGIC = 12582912.0           # 1.5*2^23 : fp32 RNE rounding magic
QS = 1024.0                  # 2^10
QI = 1.0 / 1024.0
QB2 = -12288.0               # -MAGIC * 2^-10
QMAX = 32767.0 / 1024.0
QMIN = -32.0
GEN_EPS = 1e-7
BN_EPS = 1e-5
NCORES = 8

LAST_RESULTS = None          # BassKernelResults of the most recent run (for test.py)


class Cfg:
    def __init__(self, N, E, G, XD=8, ED=4, C=32, L=4, ncores=NCORES,
                 use_collectives=True, use_shared=True, gather_k=None,
                 no_indirect=False, gather_plain=True, n_layers=None):
        self.N, self.E, self.G = N, E, G
        self.XD, self.ED, self.C, self.L = XD, ED, C, L
        self.ncores = ncores
        self.use_collectives = use_collectives and ncores > 1
        self.use_shared = use_shared
        self.gather_k = gather_k
        self.no_indirect = no_indirect
        self.gather_plain = gather_plain
        self.n_layers = L if n_layers is None else n_layers
        self.TPC = (N + ncores * 128 - 1) // (ncores * 128)    # node tiles per core
        self.NB = self.TPC * 128                               # nodes per core (padded)
        self.NPAD = self.NB * ncores
        self.PG = ((G + 128) + 127) // 128 * 128               # pooled scatter rows
        self.Kg = None                                         # chunks per tile g [TPC]
        self.E_PAD = None                                      # padded edges per core


# ----------------------------------------------------------------------------
# Host-side preprocessing: sort/bucket edges, build per-core input arrays.
# ----------------------------------------------------------------------------

def preprocess(inputs, cfg):
    x = np.ascontiguousarray(np.asarray(inputs["x"], np.float32))
    ea = np.ascontiguousarray(np.asarray(inputs["edge_attr"], np.float32))
    ei = np.asarray(inputs["edge_index"]).astype(np.int64)
    batch = np.asarray(inputs["batch"]).astype(np.int64)
    N, E, G = cfg.N, cfg.E, cfg.G
    XD, ED, C, L = cfg.XD, cfg.ED, cfg.C, cfg.L
    TPC, NB = cfg.TPC, cfg.NB
    NC_ = cfg.ncores

    assert np.abs(x).max() < 16.0, "x out of safe no-clip range"
    assert np.abs(ea).max() < 16.0, "edge_attr out of safe no-clip range"

    src, dst = ei[0], ei[1]
    order = np.argsort(dst, kind="stable")
    src_s = src[order]
    dst_s = dst[order]
    ea_s = ea[order]

    ntiles = NC_ * TPC
    bnd = np.searchsorted(dst_s, np.arange(ntiles + 1) * 128)
    cnt = np.diff(bnd)
    K_t = np.maximum((cnt + 127) // 128, 1).reshape(NC_, TPC)
    Kg = K_t.max(axis=0).astype(np.int64)                     # [TPC] shared
    E_PAD = int(Kg.sum()) * 128
    Foff = np.concatenate([[0], np.cumsum(Kg) * 128])         # flat slot offset per g

    idx_a = np.zeros((NC_, E_PAD), np.int32)
    dloc_a = np.full((NC_, E_PAD), -1.0, np.float32)
    eaT_a = np.zeros((NC_, ED + 1, E_PAD), np.float32)
    eaT_a[:, ED, :] = 1.0
    for c in range(NC_):
        for g in range(TPC):
            t = c * TPC + g
            m = int(cnt[t])
            if m == 0:
                continue
            K = int(Kg[g])
            f = np.arange(128 * K)
            p, j = f // K, f % K
            es = j * 128 + p                    # edge slot in chunk-major order
            v = es < m
            rows = bnd[t] + es[v]
            fo = int(Foff[g])
            idx_a[c, fo + f[v]] = src_s[rows]
            dloc_a[c, fo + f[v]] = (dst_s[rows] - t * 128).astype(np.float32)
            eaT_a[c, :ED, fo + f[v]] = ea_s[rows]

    xT_a = np.zeros((NC_, XD + 1, NB), np.float32)
    xT_a[:, XD, :] = 1.0
    bloc_a = np.full((NC_, NB), -1.0, np.float32)
    first_g = np.zeros(NC_, np.int64)
    for c in range(NC_):
        lo, hi = c * NB, min((c + 1) * NB, N)
        xT_a[c, :XD, : hi - lo] = x[lo:hi].T
        first_g[c] = batch[lo]
        assert batch[hi - 1] - batch[lo] < 128, "graph window exceeds 128"
        bloc_a[c, : hi - lo] = batch[lo:hi].astype(np.float32)

    prow_a = (first_g[:, None] + np.arange(128)[None, :]).astype(np.int32)
    assert prow_a.max() < cfg.PG
    cnt_g = np.bincount(batch, minlength=G).astype(np.float32)
    cnt_inv = np.zeros(cfg.PG, np.float32)
    cnt_inv[:G] = np.float32(1.0) / np.maximum(cnt_g, np.float32(1.0))

    cfg.Kg = [int(k) for k in Kg]
    cfg.E_PAD = E_PAD

    def f32(a):
        return np.ascontiguousarray(np.asarray(a, np.float32))

    shared = dict(
        W_node=f32(inputs["W_node"]), b_node=f32(inputs["b_node"]).reshape(1, C),
        W_edge=f32(inputs["W_edge"]), b_edge=f32(inputs["b_edge"]).reshape(1, C),
        bnn_g=f32(inputs["bnn_g"]).reshape(1, C), bnn_b=f32(inputs["bnn_b"]).reshape(1, C),
        bnn_m=f32(inputs["bnn_m"]).reshape(1, C), bnn_v=f32(inputs["bnn_v"]).reshape(1, C),
        bne_g=f32(inputs["bne_g"]).reshape(1, C), bne_b=f32(inputs["bne_b"]).reshape(1, C),
        bne_m=f32(inputs["bne_m"]).reshape(1, C), bne_v=f32(inputs["bne_v"]).reshape(1, C),
        t=f32(inputs["t"]).reshape(1, L),
        W1=f32(inputs["W1"]), b1=f32(inputs["b1"]),
        bn1_g=f32(inputs["bn1_g"]), bn1_b=f32(inputs["bn1_b"]),
        bn1_m=f32(inputs["bn1_m"]), bn1_v=f32(inputs["bn1_v"]),
        W2=f32(inputs["W2"]), b2=f32(inputs["b2"]),
        W_out=f32(inputs["W_out"]), b_out=f32(inputs["b_out"]).reshape(1, 1),
        cnt_inv=cnt_inv,
    )
    in_maps = []
    for c in range(NC_):
        im = dict(shared)
        im.update(
            xT=xT_a[c], eaT=eaT_a[c].reshape(-1), gidx=idx_a[c], dloc=dloc_a[c],
            bloc=bloc_a[c], prow=prow_a[c],
        )
        in_maps.append(im)
    return in_maps


# ----------------------------------------------------------------------------
# Device program.
# ----------------------------------------------------------------------------

def emit_q(nc, ap, pre_bias_ap=None, clip=True):
    """In-place fake quantization of `ap` (fp32): q(x) (+fused bias if given).

    If pre_bias_ap is given it must hold (1024*bias_q + MAGIC) per partition and
    the op computes q(x + bias_q)."""
    if pre_bias_ap is None:
        nc.scalar.activation(ap, ap, ACTF.Copy, bias=MAGIC, scale=QS)
    else:
        nc.scalar.activation(ap, ap, ACTF.Identity, bias=pre_bias_ap, scale=QS)
    nc.scalar.activation(ap, ap, ACTF.Copy, bias=QB2, scale=QI)
    if clip:
        nc.vector.tensor_scalar(ap, ap, QMAX, QMIN, AL.min, AL.max)


def build(cfg):
    C, L, TPC, NB = cfg.C, cfg.L, cfg.TPC, cfg.NB
    XD, ED, G, PG = cfg.XD, cfg.ED, cfg.G, cfg.PG
    NPAD, E_PAD, Kg = cfg.NPAD, cfg.E_PAD, cfg.Kg
    C2 = 2 * C
    RG = [list(range(cfg.ncores))]
    SHARED = "Shared" if (cfg.use_shared and cfg.use_collectives) else "Local"

    nc = bacc.Bacc("TRN2", target_bir_lowering=False, debug=False,
                   enable_asserts=False, num_devices=cfg.ncores)

    # ---- kernel I/O ----
    d_xT = nc.dram_tensor("xT", [XD + 1, NB], F32, kind="ExternalInput")
    d_eaT = nc.dram_tensor("eaT", [(ED + 1) * E_PAD], F32, kind="ExternalInput")
    d_gidx = nc.dram_tensor("gidx", [E_PAD], I32, kind="ExternalInput")
    d_dloc = nc.dram_tensor("dloc", [E_PAD], F32, kind="ExternalInput")
    d_bloc = nc.dram_tensor("bloc", [NB], F32, kind="ExternalInput")
    d_prow = nc.dram_tensor("prow", [128], I32, kind="ExternalInput")
    d_cntinv = nc.dram_tensor("cnt_inv", [PG], F32, kind="ExternalInput")
    d_Wn = nc.dram_tensor("W_node", [XD, C], F32, kind="ExternalInput")
    d_bn_ = nc.dram_tensor("b_node", [1, C], F32, kind="ExternalInput")
    d_We = nc.dram_tensor("W_edge", [ED, C], F32, kind="ExternalInput")
    d_be = nc.dram_tensor("b_edge", [1, C], F32, kind="ExternalInput")
    d_bnr = {k: nc.dram_tensor(k, [1, C], F32, kind="ExternalInput")
             for k in ["bnn_g", "bnn_b", "bnn_m", "bnn_v",
                       "bne_g", "bne_b", "bne_m", "bne_v"]}
    d_t = nc.dram_tensor("t", [1, L], F32, kind="ExternalInput")
    d_W1 = nc.dram_tensor("W1", [L, C, C2], F32, kind="ExternalInput")
    d_b1 = nc.dram_tensor("b1", [L, C2], F32, kind="ExternalInput")
    d_bn1 = {k: nc.dram_tensor(k, [L, C2], F32, kind="ExternalInput")
             for k in ["bn1_g", "bn1_b", "bn1_m", "bn1_v"]}
    d_W2 = nc.dram_tensor("W2", [L, C2, C], F32, kind="ExternalInput")
    d_b2 = nc.dram_tensor("b2", [L, C], F32, kind="ExternalInput")
    d_Wo = nc.dram_tensor("W_out", [C, 1], F32, kind="ExternalInput")
    d_bo = nc.dram_tensor("b_out", [1, 1], F32, kind="ExternalInput")
    d_out = nc.dram_tensor("out", [G, 1], F32, kind="ExternalOutput")
    d_hdbg = nc.dram_tensor("h_dbg", [NPAD, C], F32, kind="ExternalOutput")

    # ---- inline constants ----
    eye = np.eye(128, dtype=np.float32)
    iota = np.tile(np.arange(128, dtype=np.float32), (128, 1))
    iota4_np = np.tile(np.arange(128, dtype=np.float32), (128, 4, 1))
    ones_np = np.ones((1, 128), np.float32)
    c_eye = nc.inline_tensor(eye, "c_eye")
    c_iota = nc.inline_tensor(iota, "c_iota")
    c_iota4 = nc.inline_tensor(iota4_np, "c_iota4")
    NW = PG // 128                                   # pooling windows
    iota5_np = (np.tile(np.arange(128, dtype=np.float32), (128, NW, 1))
                + (np.arange(NW, dtype=np.float32) * 128)[None, :, None])
    c_iota5 = nc.inline_tensor(iota5_np, "c_iota5")
    c_ones = nc.inline_tensor(ones_np, "c_ones")

    with tile.TileContext(nc) as tc:
        with (
            tc.tile_pool(name="dram", bufs=1, space="DRAM") as dpool,
            tc.tile_pool(name="const", bufs=1) as cp,
        ):
            # ---- internal DRAM ----
            h_locA = dpool.tile([NB, C], F32, name="h_locA")
            h_locB = dpool.tile([NB, C], F32, name="h_locB")
            h_fulls = [dpool.tile([NPAD, C], F32, addr_space=SHARED,
                                  name=f"h_full_{l}") for l in range(L)]
            e_dram = dpool.tile([E_PAD * C], F32, name="e_dram")
            xq_dram = dpool.tile([(XD + 1) * NB], F32, name="xq_dram")
            eaq_dram = dpool.tile([(ED + 1) * E_PAD], F32, name="eaq_dram")
            pool_glob = dpool.tile([PG, C], F32, name="pool_glob")
            pool_red = dpool.tile([PG, C], F32, addr_space=SHARED, name="pool_red")

            # ---- constants to SBUF ----
            ident = cp.tile([128, 128], F32, name="ident")
            nc.sync.dma_start(ident[:, :], c_eye[:, :])
            iota1 = cp.tile([128, 128], F32, name="iota1")
            nc.sync.dma_start(iota1[:, :], c_iota[:, :])
            iota4 = cp.tile([128, 4, 128], F32, name="iota4")
            nc.sync.dma_start(iota4[:, :, :], c_iota4[:, :, :])
            iota5 = cp.tile([128, NW, 128], F32, name="iota5")
            nc.sync.dma_start(iota5[:, :, :], c_iota5[:, :, :])
            pacc = cp.tile([128, NW, C], F32, name="pacc")
            nc.gpsimd.memset(pacc[:, :, :], 0.0)
            onesr = cp.tile([1, 128], F32, name="onesr")
            nc.sync.dma_start(onesr[:, :], c_ones[:, :])

            # ---- parameter prep ----
            rhs_node = cp.tile([XD + 1, C], F32, name="rhs_node")
            nc.sync.dma_start(rhs_node[:XD, :], d_Wn[:, :])
            nc.sync.dma_start(rhs_node[XD:XD + 1, :], d_bn_[:, :])
            emit_q(nc, rhs_node[:, :])
            rhs_edge = cp.tile([ED + 1, C], F32, name="rhs_edge")
            nc.sync.dma_start(rhs_edge[:ED, :], d_We[:, :])
            nc.sync.dma_start(rhs_edge[ED:ED + 1, :], d_be[:, :])
            emit_q(nc, rhs_edge[:, :])

            def bn_rows(pref):
                g_ = cp.tile([1, C], F32, name=pref + "_g")
                b_ = cp.tile([1, C], F32, name=pref + "_b")
                m_ = cp.tile([1, C], F32, name=pref + "_m")
                sc = cp.tile([1, C], F32, name=pref + "_sc")
                bi = cp.tile([1, C], F32, name=pref + "_bi")
                nc.sync.dma_start(g_[:, :], d_bnr[pref + "_g"][:, :])
                nc.sync.dma_start(b_[:, :], d_bnr[pref + "_b"][:, :])
                nc.sync.dma_start(m_[:, :], d_bnr[pref + "_m"][:, :])
                nc.sync.dma_start(sc[:, :], d_bnr[pref + "_v"][:, :])
                nc.vector.tensor_scalar(sc[:, :], sc[:, :], BN_EPS, None, AL.add)
                nc.scalar.activation(sc[:, :], sc[:, :], ACTF.Sqrt)
                nc.vector.reciprocal(sc[:, :], sc[:, :])
                nc.vector.tensor_tensor(sc[:, :], sc[:, :], g_[:, :], op=AL.mult)
                nc.vector.tensor_tensor(bi[:, :], m_[:, :], sc[:, :], op=AL.mult)
                nc.vector.tensor_tensor(bi[:, :], b_[:, :], bi[:, :], op=AL.subtract)
                return sc, bi

            scN, biN = bn_rows("bnn")
            scE, biE = bn_rows("bne")

            def replicate4(row, nm, pool):
                ps = pool.tile([128, C], F32, name="rep_ps", tag="encp")
                nc.tensor.matmul(ps[:, :], lhsT=onesr[:, :], rhs=row[:, :],
                                 start=True, stop=True)
                out4 = cp.tile([128, 4 * C], F32, name=nm)
                for q in range(4):
                    nc.vector.tensor_copy(out4[:, q * C:(q + 1) * C], ps[:, :])
                return out4

            W1q, bias1, sc1, bi1, W2q, bias2 = [], [], [], [], [], []
            for l in range(L):
                w1 = cp.tile([C, C2], F32, name=f"W1q_{l}")
                nc.sync.dma_start(w1[:, :], d_W1[l, :, :])
                emit_q(nc, w1[:, :])
                W1q.append(w1)
                b1t = cp.tile([C2, 1], F32, name=f"bias1_{l}")
                nc.sync.dma_start(b1t[:, :], d_b1[l:l + 1, :].rearrange("a b -> b a"))
                emit_q(nc, b1t[:, :])
                nc.vector.tensor_scalar(b1t[:, :], b1t[:, :], QS, MAGIC, AL.mult, AL.add)
                bias1.append(b1t)

                g1 = cp.tile([C2, 1], F32, name=f"g1_{l}")
                bb1 = cp.tile([C2, 1], F32, name=f"bb1_{l}")
                m1 = cp.tile([C2, 1], F32, name=f"m1_{l}")
                s1 = cp.tile([C2, 1], F32, name=f"sc1_{l}")
                i1 = cp.tile([C2, 1], F32, name=f"bi1_{l}")
                nc.sync.dma_start(g1[:, :], d_bn1["bn1_g"][l:l + 1, :].rearrange("a b -> b a"))
                nc.sync.dma_start(bb1[:, :], d_bn1["bn1_b"][l:l + 1, :].rearrange("a b -> b a"))
                nc.sync.dma_start(m1[:, :], d_bn1["bn1_m"][l:l + 1, :].rearrange("a b -> b a"))
                nc.sync.dma_start(s1[:, :], d_bn1["bn1_v"][l:l + 1, :].rearrange("a b -> b a"))
                nc.vector.tensor_scalar(s1[:, :], s1[:, :], BN_EPS, None, AL.add)
                nc.scalar.activation(s1[:, :], s1[:, :], ACTF.Sqrt)
                nc.vector.reciprocal(s1[:, :], s1[:, :])
                nc.vector.tensor_tensor(s1[:, :], s1[:, :], g1[:, :], op=AL.mult)
                nc.vector.tensor_tensor(i1[:, :], m1[:, :], s1[:, :], op=AL.mult)
                nc.vector.tensor_tensor(i1[:, :], bb1[:, :], i1[:, :], op=AL.subtract)
                sc1.append(s1)
                bi1.append(i1)

                w2 = cp.tile([C2, C], F32, name=f"W2q_{l}")
                nc.sync.dma_start(w2[:, :], d_W2[l, :, :])
                emit_q(nc, w2[:, :])
                W2q.append(w2)
                b2t = cp.tile([C, 1], F32, name=f"bias2_{l}")
                nc.sync.dma_start(b2t[:, :], d_b2[l:l + 1, :].rearrange("a b -> b a"))
                emit_q(nc, b2t[:, :])
                nc.vector.tensor_scalar(b2t[:, :], b2t[:, :], QS, MAGIC, AL.mult, AL.add)
                bias2.append(b2t)

            Woq = cp.tile([C, 1], F32, name="Woq")
            nc.sync.dma_start(Woq[:, :], d_Wo[:, :])
            emit_q(nc, Woq[:, :])
            biaso = cp.tile([1, 1], F32, name="biaso")
            nc.sync.dma_start(biaso[:, :], d_bo[:, :])
            emit_q(nc, biaso[:, :])
            nc.vector.tensor_scalar(biaso[:, :], biaso[:, :], QS, MAGIC, AL.mult, AL.add)


            # ---- encoders ----
            def q_pass(src_flat, dst_flat, total, pool):
                per = total // 128
                assert total % 128 == 0
                W = min(per, 4096)
                n = (per + W - 1) // W
                sv = src_flat.rearrange("(p q) -> p q", p=128)
                dv = dst_flat.rearrange("(p q) -> p q", p=128)
                for i in range(n):
                    w = min(W, per - i * W)
                    tl = pool.tile([128, W], F32, tag="qpass", name="qpass")
                    nc.sync.dma_start(tl[:, :w], sv[:, i * W:i * W + w])
                    nc.scalar.activation(tl[:, :w], tl[:, :w], ACTF.Copy,
                                         bias=MAGIC, scale=QS)
                    nc.scalar.activation(tl[:, :w], tl[:, :w], ACTF.Copy,
                                         bias=QB2, scale=QI)
                    nc.sync.dma_start(dv[:, i * W:i * W + w], tl[:, :w])

            with (
                tc.tile_pool(name="enc", bufs=2) as enc,
                tc.tile_pool(name="encx", bufs=1) as encx,
                tc.tile_pool(name="encps", bufs=2, space="PSUM") as enc_ps,
            ):
                scN4 = replicate4(scN, "scN4", enc_ps)
                biN4 = replicate4(biN, "biN4", enc_ps)
                scE4 = replicate4(scE, "scE4", enc_ps)
                biE4 = replicate4(biE, "biE4", enc_ps)

                t_sb = cp.tile([1, L], F32, name="t_sb")
                nc.sync.dma_start(t_sb[:, :], d_t[:, :])
                t_ps = enc_ps.tile([128, L], F32, name="t_ps", tag="encp")
                nc.tensor.matmul(t_ps[:, :], lhsT=onesr[:, :], rhs=t_sb[:, :],
                                 start=True, stop=True)
                t_bc = cp.tile([128, L], F32, name="t_bc")
                nc.vector.tensor_copy(t_bc[:, :], t_ps[:, :])
                teps_bc = cp.tile([128, L], F32, name="teps_bc")
                nc.vector.tensor_scalar(teps_bc[:, :], t_bc[:, :], GEN_EPS, None,
                                        AL.mult)

                # node encoder
                q_pass(d_xT[:, :].rearrange("a b -> (a b)"), xq_dram[:], (XD + 1) * NB, enc)
                xseg = encx.tile([XD + 1, NB], F32, name="xseg")
                nc.sync.dma_start(
                    xseg[:, :], xq_dram[:].rearrange("(r e) -> r e", r=XD + 1))
                for b in range(0, TPC, 4):
                    gs = min(4, TPC - b)
                    ep = enc_ps.tile([128, 4 * C], F32, name="encp", tag="encp")
                    for q in range(gs):
                        nc.tensor.matmul(
                            ep[:, q * C:(q + 1) * C],
                            lhsT=xseg[:, (b + q) * 128:(b + q + 1) * 128],
                            rhs=rhs_node[:, :], start=True, stop=True)
                    es = enc.tile([128, 4 * C], F32, name="encs", tag="encs")
                    nc.scalar.activation(es[:, :gs * C], ep[:, :gs * C], ACTF.Copy,
                                         bias=MAGIC, scale=QS)
                    nc.scalar.activation(es[:, :gs * C], es[:, :gs * C], ACTF.Copy,
                                         bias=QB2, scale=QI)
                    nc.vector.tensor_scalar(es[:, :gs * C], es[:, :gs * C],
                                            QMAX, QMIN, AL.min, AL.max)
                    nc.vector.tensor_tensor(es[:, :gs * C], es[:, :gs * C],
                                            scN4[:, :gs * C], op=AL.mult)
                    nc.vector.tensor_tensor(es[:, :gs * C], es[:, :gs * C],
                                            biN4[:, :gs * C], op=AL.add)
                    nc.sync.dma_start(
                        h_locA[b * 128:(b + gs) * 128, :]
                        .rearrange("(t p) c -> p t c", p=128),
                        es[:, :gs * C].rearrange("p (t c) -> p t c", c=C))

                # edge encoder
                q_pass(d_eaT[:], eaq_dram[:], (ED + 1) * E_PAD, enc)
                eav = eaq_dram[:].rearrange("(r e) -> r e", r=ED + 1)
                n_ch = E_PAD // 128
                SEGC = 32                       # chunks per staged segment
                for s0 in range(0, n_ch, SEGC):
                    sc_ = min(SEGC, n_ch - s0)
                    eseg = enc.tile([ED + 1, SEGC * 128], F32, name="eseg", tag="eseg")
                    nc.sync.dma_start(eseg[:, :sc_ * 128],
                                      eav[:, s0 * 128:(s0 + sc_) * 128])
                    for b in range(0, sc_, 4):
                        gs = min(4, sc_ - b)
                        ep = enc_ps.tile([128, 4 * C], F32, name="encp", tag="encp")
                        for q in range(gs):
                            nc.tensor.matmul(
                                ep[:, q * C:(q + 1) * C],
                                lhsT=eseg[:, (b + q) * 128:(b + q + 1) * 128],
                                rhs=rhs_edge[:, :], start=True, stop=True)
                        es = enc.tile([128, 4 * C], F32, name="encs2", tag="encs")
                        nc.scalar.activation(es[:, :gs * C], ep[:, :gs * C], ACTF.Copy,
                                             bias=MAGIC, scale=QS)
                        nc.scalar.activation(es[:, :gs * C], es[:, :gs * C], ACTF.Copy,
                                             bias=QB2, scale=QI)
                        nc.vector.tensor_scalar(es[:, :gs * C], es[:, :gs * C],
                                                QMAX, QMIN, AL.min, AL.max)
                        nc.vector.tensor_tensor(es[:, :gs * C], es[:, :gs * C],
                                                scE4[:, :gs * C], op=AL.mult)
                        nc.vector.tensor_tensor(es[:, :gs * C], es[:, :gs * C],
                                                biE4[:, :gs * C], op=AL.add)
                        r0 = (s0 + b) * 128
                        nc.sync.dma_start(
                            e_dram[r0 * C:(r0 + gs * 128) * C]
                            .rearrange("(t p c) -> p t c", p=128, c=C),
                            es[:, :gs * C].rearrange("p (t c) -> p t c", c=C))

                # first AllGather
                if cfg.use_collectives:
                    nc.gpsimd.collective_compute(
                        "AllGather", AL.bypass, replica_groups=RG,
                        ins=[h_locA[:, :]], outs=[h_fulls[0][:, :]])
                else:
                    for b_ in range(cfg.ncores):
                        nc.sync.dma_start(h_fulls[0][b_ * NB:(b_ + 1) * NB, :],
                                          h_locA[:, :])

            # ---- layers ----
            with (
                tc.tile_pool(name="edge", bufs=3) as epool,
                tc.tile_pool(name="node", bufs=2) as npool,
                tc.tile_pool(name="eps", bufs=2, space="PSUM") as ps_edge,
                tc.tile_pool(name="mlp1", bufs=2, space="PSUM") as ps_z1,
                tc.tile_pool(name="mlp2", bufs=1, space="PSUM") as ps_z2,
                tc.tile_pool(name="tr", bufs=2, space="PSUM") as ps_tr,
                tc.tile_pool(name="poolps", bufs=1, space="PSUM") as ps_pool,
            ):
              Foff = np.concatenate([[0], np.cumsum(np.asarray(Kg)) * 128]).astype(int)

              for l in range(min(L, cfg.n_layers)):
                  h_in = h_locA if l % 2 == 0 else h_locB
                  h_out = h_locB if l % 2 == 0 else h_locA
                  last = l == L - 1

                  hog = None
                  h2qT = None
                  for g in range(TPC):
                      K = Kg[g]
                      F = int(Foff[g])
                      t = g % 4
                      if t == 0:
                          gs = min(4, TPC - g)
                          hog = npool.tile([128, 4, C], F32, name="hog", tag="hog")
                          h2qT = npool.tile([C, 512], F32, name="h2qT", tag="h2qT")

                      # --- edge phase ---
                      et = epool.tile([128, K, C], F32, name="et", tag="et",
                                      padded_shape=[128, max(Kg), C])
                      nc.sync.dma_start(
                          et[:, :, :],
                          e_dram[F * C:(F + 128 * K) * C]
                          .rearrange("(p k c) -> p k c", p=128, k=K))
                      idxt = epool.tile([128, K], I32, name="idxt", tag="idxt",
                                        padded_shape=[128, max(Kg)])
                      nc.sync.dma_start(
                          idxt[:, :],
                          d_gidx[F:F + 128 * K].rearrange("(p k) -> p k", p=128))
                      dlt = epool.tile([128, K], F32, name="dlt", tag="dlt",
                                       padded_shape=[128, max(Kg)])
                      nc.sync.dma_start(
                          dlt[:, :],
                          d_dloc[F:F + 128 * K].rearrange("(p k) -> p k", p=128))
                      # gather h[src] and accumulate onto e -> et = h_src + e
                      # (HW indirect DMA consumes ONE index per partition per
                      #  instruction, so gather chunk-by-chunk: [128,1] idx.)
                      if not cfg.no_indirect:
                          if cfg.gather_plain:
                              hsg = epool.tile([128, K, C], F32, name="hsg",
                                               tag="hsg",
                                               padded_shape=[128, max(Kg), C])
                              for j in range(K):
                                  nc.gpsimd.indirect_dma_start(
                                      out=hsg[:, j, :], out_offset=None,
                                      in_=h_fulls[l][:, :],
                                      in_offset=IndirectOffsetOnAxis(
                                          ap=idxt[:, j:j + 1], axis=0))
                              nc.vector.tensor_tensor(et[:, :, :], et[:, :, :],
                                                      hsg[:, :, :], op=AL.add)
                          else:
                              for j in range(K):
                                  nc.gpsimd.indirect_dma_start(
                                      out=et[:, j, :], out_offset=None,
                                      in_=h_fulls[l][:, :],
                                      in_offset=IndirectOffsetOnAxis(
                                          ap=idxt[:, j:j + 1], axis=0),
                                      compute_op=AL.add)
                      # r = relu(h_src + e)
                      nc.scalar.activation(et[:, :, :], et[:, :, :], ACTF.Relu)
                      exmex = epool.tile([128, K, C2], F32, name="exmex", tag="exmex",
                                         padded_shape=[128, max(Kg), C2])
                      # ex = exp(t_l * r + t_l*eps)
                      nc.scalar.activation(exmex[:, :, C:C2], et[:, :, :], ACTF.Exp,
                                           bias=teps_bc[:, l:l + 1],
                                           scale=t_bc[:, l:l + 1])
                      # m = r + eps
                      nc.vector.tensor_scalar(et[:, :, :], et[:, :, :], GEN_EPS,
                                              None, AL.add)
                      # ex*m
                      nc.vector.tensor_tensor(exmex[:, :, 0:C], exmex[:, :, C:C2],
                                              et[:, :, :], op=AL.mult)

                      eps_t = ps_edge.tile([128, C2], F32, name="eps_t", tag="eps_t")
                      for j0 in range(0, K, 4):
                          jj = min(4, K - j0)
                          oh4 = epool.tile([128, 4, 128], F32, name="oh4", tag="oh4")
                          nc.vector.tensor_tensor(
                              oh4[:, :jj, :],
                              dlt[:, j0:j0 + jj].to_broadcast([128, jj, 128]),
                              iota4[:, :jj, :], op=AL.is_equal)
                          for q in range(jj):
                              j = j0 + q
                              nc.tensor.matmul(
                                  eps_t[:, :], lhsT=oh4[:, q, :],
                                  rhs=exmex[:, j, :],
                                  start=(j == 0), stop=(j == K - 1))

                      # agg = num / max(den, 1e-16);  h2 = h_own + agg
                      nc.sync.dma_start(hog[:, t, :], h_in[g * 128:(g + 1) * 128, :])
                      dinv = npool.tile([128, C], F32, name="dinv", tag="dinv")
                      nc.vector.tensor_scalar(dinv[:, :], eps_t[:, C:C2], 1e-16,
                                              None, AL.max)
                      nc.vector.reciprocal(dinv[:, :], dinv[:, :])
                      h2 = npool.tile([128, C], F32, name="h2", tag="h2")
                      nc.vector.tensor_tensor(h2[:, :], eps_t[:, 0:C], dinv[:, :],
                                              op=AL.mult)
                      nc.vector.tensor_tensor(h2[:, :], h2[:, :], hog[:, t, :],
                                              op=AL.add)
                      emit_q(nc, h2[:, :])
                      trp = ps_tr.tile([C, 128], F32, name="trp", tag="tr")
                      nc.tensor.transpose(trp[:, :], h2[:, :], identity=ident[:, :])
                      nc.vector.tensor_copy(h2qT[:, t * 128:(t + 1) * 128], trp[:, :])

                      # --- MLP on a full group of up to 4 node tiles ---
                      if t == gs - 1 or g == TPC - 1:
                          w = gs * 128
                          z1p = ps_z1.tile([C2, 512], F32, name="z1p", tag="z1p")
                          nc.tensor.matmul(z1p[:, :w], lhsT=W1q[l][:, :],
                                           rhs=h2qT[:, :w], start=True, stop=True)
                          z1s = npool.tile([C2, 512], F32, name="z1s", tag="z1s")
                          nc.scalar.activation(z1s[:, :w], z1p[:, :w], ACTF.Identity,
                                               bias=bias1[l][:, :], scale=QS)
                          nc.scalar.activation(z1s[:, :w], z1s[:, :w], ACTF.Copy,
                                               bias=QB2, scale=QI)
                          nc.vector.tensor_scalar(z1s[:, :w], z1s[:, :w], QMAX, QMIN,
                                                  AL.min, AL.max)
                          nc.scalar.activation(z1s[:, :w], z1s[:, :w], ACTF.Relu,
                                               bias=bi1[l][:, :], scale=sc1[l][:, :])
                          nc.scalar.activation(z1s[:, :w], z1s[:, :w], ACTF.Copy,
                                               bias=MAGIC, scale=QS)
                          nc.scalar.activation(z1s[:, :w], z1s[:, :w], ACTF.Copy,
                                               bias=QB2, scale=QI)
                          nc.vector.tensor_scalar(z1s[:, :w], z1s[:, :w], QMAX, QMIN,
                                                  AL.min, AL.max)
                          z2p = ps_z2.tile([C, 512], F32, name="z2p", tag="z2p")
                          nc.tensor.matmul(z2p[:, :w], lhsT=W2q[l][:, :],
                                           rhs=z1s[:, :w], start=True, stop=True)
                          z2s = npool.tile([C, 512], F32, name="z2s", tag="z2s")
                          nc.scalar.activation(z2s[:, :w], z2p[:, :w], ACTF.Identity,
                                               bias=bias2[l][:, :], scale=QS)
                          nc.scalar.activation(z2s[:, :w], z2s[:, :w], ACTF.Copy,
                                               bias=QB2, scale=QI)
                          nc.vector.tensor_scalar(z2s[:, :w], z2s[:, :w], QMAX, QMIN,
                                                  AL.min, AL.max)
                          hnext = npool.tile([128, 4, C], F32, name="hnext", tag="hnext")
                          g0 = g - gs + 1
                          for q in range(gs):
                              trq = ps_tr.tile([128, C], F32, name="trq", tag="tr")
                              nc.tensor.transpose(trq[:, :],
                                                  z2s[:, q * 128:(q + 1) * 128],
                                                  identity=ident[0:C, 0:C])
                              nc.vector.tensor_tensor(hnext[:, q, :], trq[:, :],
                                                      hog[:, q, :], op=AL.add)
                              if last:
                                  blt = npool.tile([128, 1], F32, name="blt", tag="blt")
                                  nc.sync.dma_start(
                                      blt[:, :],
                                      d_bloc[(g0 + q) * 128:(g0 + q + 1) * 128]
                                      .rearrange("(p one) -> p one", one=1))
                                  ohp = npool.tile([128, NW, 128], F32, name="ohp",
                                                   tag="ohp")
                                  nc.vector.tensor_tensor(
                                      ohp[:, :, :],
                                      blt[:, :].to_broadcast([128, NW, 128]),
                                      iota5[:, :, :], op=AL.is_equal)
                                  for wi in range(NW):
                                      pps = ps_pool.tile([128, C], F32, name="pps",
                                                         tag="pps")
                                      nc.tensor.matmul(
                                          pps[:, :], lhsT=ohp[:, wi, :],
                                          rhs=hnext[:, q, :],
                                          start=True, stop=True)
                                      nc.vector.tensor_tensor(
                                          pacc[:, wi, :], pacc[:, wi, :], pps[:, :],
                                          op=AL.add)
                          if not last:
                              nc.sync.dma_start(
                                  h_out[g0 * 128:(g0 + gs) * 128, :]
                                  .rearrange("(t p) c -> p t c", p=128),
                                  hnext[:, :gs, :])

                  if not last:
                      if cfg.use_collectives:
                          nc.gpsimd.collective_compute(
                              "AllGather", AL.bypass, replica_groups=RG,
                              ins=[h_out[:, :]], outs=[h_fulls[l + 1][:, :]])
                      else:
                          for b_ in range(cfg.ncores):
                              nc.sync.dma_start(
                                  h_fulls[l + 1][b_ * NB:(b_ + 1) * NB, :],
                                  h_out[:, :])

              if cfg.n_layers < L:
                  nl = cfg.n_layers
                  hf = h_fulls[min(nl, L - 1)]
                  for b_ in range(NPAD // 128):
                      dbg_t = npool.tile([128, C], F32, name="dbg_t", tag="dbg_t")
                      nc.sync.dma_start(dbg_t[:, :],
                                        hf[b_ * 128:(b_ + 1) * 128, :])
                      nc.sync.dma_start(d_hdbg[b_ * 128:(b_ + 1) * 128, :],
                                        dbg_t[:, :])
                  return nc

              # ---- pooling: write window partials, AllReduce, output head ----
              nc.sync.dma_start(
                  pool_glob[:, :].rearrange("(w p) c -> p w c", p=128),
                  pacc[:, :, :])
              if cfg.use_collectives:
                  nc.gpsimd.collective_compute(
                      "AllReduce", AL.add, replica_groups=RG,
                      ins=[pool_glob[:, :]], outs=[pool_red[:, :]])
              else:
                  nc.sync.dma_start(pool_red[:, :], pool_glob[:, :])

              n_out_tiles = (G + 127) // 128
              for i in range(n_out_tiles):
                  w = min(128, G - i * 128)
                  pt = npool.tile([128, C], F32, name="pt", tag="pt")
                  nc.sync.dma_start(pt[:w, :], pool_red[i * 128:i * 128 + w, :])
                  civ = npool.tile([128, 1], F32, name="civ", tag="civ")
                  nc.sync.dma_start(civ[:w, :],
                                    d_cntinv[i * 128:i * 128 + w].rearrange("(p one) -> p one", one=1))
                  nc.vector.tensor_scalar(pt[:w, :], pt[:w, :], civ[:w, :], None, AL.mult)
                  emit_q(nc, pt[:w, :])
                  trh = ps_tr.tile([C, 128], F32, name="trh", tag="tr")
                  nc.tensor.transpose(trh[:, :w], pt[:w, :], identity=ident[:w, :w])
                  hts = npool.tile([C, 128], F32, name="hts", tag="hts")
                  nc.vector.tensor_copy(hts[:, :w], trh[:, :w])
                  op_ = ps_z2.tile([1, 128], F32, name="op_", tag="z2p")
                  nc.tensor.matmul(op_[:, :w], lhsT=Woq[:, :], rhs=hts[:, :w],
                                   start=True, stop=True)
                  osb = npool.tile([1, 128], F32, name="osb", tag="osb")
                  nc.scalar.activation(osb[:, :w], op_[:, :w], ACTF.Identity,
                                       bias=biaso[:, :], scale=QS)
                  nc.scalar.activation(osb[:, :w], osb[:, :w], ACTF.Copy,
                                       bias=QB2, scale=QI)
                  nc.vector.tensor_scalar(osb[:, :w], osb[:, :w], QMAX, QMIN,
                                          AL.min, AL.max)
                  nc.scalar.activation(osb[:, :w], osb[:, :w], ACTF.Sigmoid)
                  nc.scalar.activation(osb[:, :w], osb[:, :w], ACTF.Copy,
                                       bias=MAGIC, scale=QS)
                  nc.scalar.activation(osb[:, :w], osb[:, :w], ACTF.Copy,
                                       bias=QB2, scale=QI)
                  nc.sync.dma_start(
                      d_out[i * 128:i * 128 + w, :].rearrange("w one -> one w"),
                      osb[:, :w])

    return nc


# ----------------------------------------------------------------------------
# Entry point.
# ----------------------------------------------------------------------------

def run(inputs, cfg, **run_kwargs):
    global LAST_RESULTS
    in_maps = preprocess(inputs, cfg)
    nc = build(cfg)
    if not nc.is_finalized():
        nc.finalize()
    res = run_bass_kernel_spmd(nc, in_maps, core_ids=list(range(cfg.ncores)),
                               **run_kwargs)
    LAST_RESULTS = res
    return res.results[0]["out"].reshape(cfg.G, 1).astype(np.float32)


def kernel(**inputs) -> np.ndarray:
    cfg = Cfg(N=100000, E=3200000, G=512, XD=8, ED=4, C=32, L=4)
    return run(inputs, cfg)



# revision 19
# speedup vs baseline: 1.1425x; 1.1425x over previous
"""Trainium2 Bass kernel: nn_BV_Model (GENConv GNN, softmax aggregation, 4 layers).

Strategy (8 NeuronCores, SPMD), v2:
  - Nodes partitioned into 8 contiguous blocks (12544/core, padded); edges
    sorted by destination and bucketed per destination node-tile (128 nodes).
  - h is replicated per-core in DRAM as fp32 [NPAD, 64] (channels padded
    32->64 so one node row is 256B) and re-AllGathered per layer.
  - The per-edge h[src] gather uses the batched SWDGE dma_gather
    (InstDMAGatherAnt): 1024 edges per instruction (vs. one 128-edge chunk
    per indirect_dma_start), with edges grouped per (8-tile supertile,
    src-range) so int16 indices stay in range (4 ranges of NPAD/4 rows).
  - Edge math: x = h_src + e; with t>=0, exp(t*relu(x)) == max(exp(t*x), 1),
    so num-term relu(x)*ex is one fused scalar_tensor_tensor op and den-term
    is one tensor_scalar max. eps=1e-7 is dropped (error ~1e-7 << tol).
  - Segment softmax reduced edges->nodes with a one-hot(dst) matmul in bf16
    accumulated in PSUM; e / exmex / one-hot are bf16 (tol 2e-2).
  - Node MLP on the tensor engine in transposed layout (fp32, exact quant).
  - Global mean pool per-core with one-hot(graph) matmuls + AllReduce.

Fake-quantization q(x) = clip(rne(x*1024), -32768, 32767)/1024 via the
round-to-nearest-even "magic number" trick (+1.5*2^23).
"""

import os
os.environ.setdefault("MYCRO_LOCAL_CACHE", "1")

import math
import numpy as np

import concourse.bacc as bacc
import concourse.tile as tile
import concourse.bass as bass
from concourse import mybir
from concourse import library_config
from concourse.bass_utils import run_bass_kernel_spmd

F32 = mybir.dt.float32
BF16 = mybir.dt.bfloat16
I16 = mybir.dt.int16
I32 = mybir.dt.int32
ACTF = mybir.ActivationFunctionType
AL = mybir.AluOpType

MAGIC = 12582912.0           # 1.5*2^23 : fp32 RNE rounding magic
QS = 1024.0                  # 2^10
QI = 1.0 / 1024.0
QB2 = -12288.0               # -MAGIC * 2^-10
QMAX = 32767.0 / 1024.0
QMIN = -32.0
# clip bounds in the y = 1024*q + MAGIC domain (pre-descale)
YMAX = QMAX * QS + MAGIC
YMIN = QMIN * QS + MAGIC
BN_EPS = 1e-5
NCORES = 8
GT = 4                       # tiles per gather/edge supertile (= MLP group)
GMAXCH = 8                   # max chunks (of 128 idx) per dma_gather instr

LAST_RESULTS = None          # BassKernelResults of the most recent run


class Cfg:
    def __init__(self, N, E, G, XD=8, ED=4, C=32, L=4, ncores=NCORES,
                 use_collectives=True, use_shared=True, gather_k=None,
                 no_indirect=False, gather_plain=True, n_layers=None):
        self.N, self.E, self.G = N, E, G
        self.XD, self.ED, self.C, self.L = XD, ED, C, L
        self.ncores = ncores
        self.use_collectives = use_collectives and ncores > 1
        self.use_shared = use_shared
        self.n_layers = L if n_layers is None else n_layers
        self.TPC = (N + ncores * 128 - 1) // (ncores * 128)    # node tiles per core
        self.NB = self.TPC * 128                               # nodes per core (padded)
        self.NPAD = self.NB * ncores
        self.NR = 4
        assert self.NPAD % self.NR == 0
        self.RSZ = self.NPAD // self.NR                        # src range rows
        assert self.RSZ <= 32767
        self.PG = ((G + 128) + 127) // 128 * 128               # pooled scatter rows
        # filled by preprocess:
        self.groups = None          # list[list[g]]
        self.chunk_meta = None      # per group: list[(t_in_group, start, stop)]
        self.instr_meta = None      # per group: list[(ch0_local, nch, r, col0)]
        self.group_F = None         # per group: global chunk offset
        self.NCHG = None            # max chunks in a group
        self.SLOTS = None
        self.SCOLS = None


# ----------------------------------------------------------------------------
# Host-side preprocessing: sort/bucket edges, build per-core input arrays.
# ----------------------------------------------------------------------------

def preprocess(inputs, cfg):
    x = np.ascontiguousarray(np.asarray(inputs["x"], np.float32))
    ea = np.ascontiguousarray(np.asarray(inputs["edge_attr"], np.float32))
    ei = np.asarray(inputs["edge_index"]).astype(np.int64)
    batch = np.asarray(inputs["batch"]).astype(np.int64)
    N, E, G = cfg.N, cfg.E, cfg.G
    XD, ED, C, L = cfg.XD, cfg.ED, cfg.C, cfg.L
    TPC, NB, NR, RSZ = cfg.TPC, cfg.NB, cfg.NR, cfg.RSZ
    NC_ = cfg.ncores

    src, dst = ei[0], ei[1]
    order = np.argsort(dst, kind="stable")
    src_s = src[order]
    dst_s = dst[order]
    ea_s = ea[order]

    ntiles = NC_ * TPC
    bnd = np.searchsorted(dst_s, np.arange(ntiles + 1) * 128)

    # per (core, tile): reorder edges by src range; count per range
    cnt = np.zeros((NC_, TPC, NR), np.int64)
    seg = {}                       # (c, g, r) -> (src_rel, dloc, ea rows)
    for c in range(NC_):
        for g in range(TPC):
            t = c * TPC + g
            b0, b1 = int(bnd[t]), int(bnd[t + 1])
            if b1 <= b0:
                continue
            s = src_s[b0:b1]
            d = dst_s[b0:b1]
            e_ = ea_s[b0:b1]
            r = s // RSZ
            o = np.argsort(r, kind="stable")
            s, d, e_, r = s[o], d[o], e_[o], r[o]
            rb = np.searchsorted(r, np.arange(NR + 1))
            for rr in range(NR):
                m = int(rb[rr + 1] - rb[rr])
                cnt[c, g, rr] = m
                if m:
                    sl = slice(int(rb[rr]), int(rb[rr + 1]))
                    seg[(c, g, rr)] = (
                        (s[sl] - rr * RSZ).astype(np.int16),
                        (d[sl] - t * 128).astype(np.float32),
                        e_[sl],
                    )

    K = (cnt.max(axis=0) + 127) // 128                         # [TPC, NR]

    # group structure + chunk/instr tables (shared across cores)
    groups = [list(range(g0, min(g0 + GT, TPC))) for g0 in range(0, TPC, GT)]
    chunk_meta, instr_meta, group_F = [], [], []
    ch_start = {}                  # (g, r) -> global chunk index
    ch = 0
    for grp in groups:
        group_F.append(ch)
        cm = []
        im = []
        # first/last chunk index (local) per tile for start/stop flags
        tile_chunks = {ti: [] for ti in range(len(grp))}
        local = 0
        runs = []
        for r in range(NR):
            run0 = local
            for ti, g in enumerate(grp):
                for _ in range(int(K[g, r])):
                    cm.append([ti, False, False])
                    tile_chunks[ti].append(local)
                    local += 1
            runs.append((run0, local - run0, r))
        for ti, lst in tile_chunks.items():
            if lst:
                cm[lst[0]][1] = True
                cm[lst[-1]][2] = True
        for (run0, n, r) in runs:
            o = 0
            while o < n:
                nch = min(GMAXCH, n - o)
                im.append([run0 + o, nch, r, 0])
                o += nch
        chunk_meta.append([tuple(e) for e in cm])
        instr_meta.append(im)
        for r in range(NR):
            for g in grp:
                ch_start[(g, r)] = ch
                ch += int(K[g, r])
    NCH_TOT = ch
    SLOTS = NCH_TOT * 128
    # idx column offsets (global, shared)
    col = 0
    maxcols = 0
    for gi in range(len(groups)):
        col0g = col
        for e in instr_meta[gi]:
            e[3] = col
            col += e[1] * 8  # nch*128/16 = nch*8 columns
        maxcols = max(maxcols, col - col0g)
        instr_meta[gi] = [tuple(e) for e in instr_meta[gi]]
    SCOLS = col

    cfg.groups, cfg.chunk_meta, cfg.instr_meta = groups, chunk_meta, instr_meta
    cfg.group_F = group_F
    cfg.NCHG = max(len(cm) for cm in chunk_meta)
    cfg.MAXCOLS = maxcols
    cfg.SLOTS, cfg.SCOLS = SLOTS, SCOLS

    # per-core slot arrays
    idxrel_a = np.zeros((NC_, SLOTS), np.int16)
    dloc_a = np.full((NC_, SLOTS), -1.0, np.float32)
    eaT_a = np.zeros((NC_, ED + 1, SLOTS), np.float32)
    eaT_a[:, ED, :] = 1.0
    for c in range(NC_):
        for g in range(TPC):
            for r in range(NR):
                m = int(cnt[c, g, r])
                if m == 0:
                    continue
                s0 = ch_start[(g, r)] * 128
                sr, dl, er = seg[(c, g, r)]
                idxrel_a[c, s0:s0 + m] = sr
                dloc_a[c, s0:s0 + m] = dl
                eaT_a[c, :ED, s0:s0 + m] = er.T

    # wrapped int16 index arrays per instruction
    idx16_a = np.zeros((NC_, 128, SCOLS), np.int16)
    for gi, grp in enumerate(groups):
        F = group_F[gi]
        for (ch0, nch, r, col0) in instr_meta[gi]:
            ni = nch * 128
            s0 = (F + ch0) * 128
            for c in range(NC_):
                flat = idxrel_a[c, s0:s0 + ni]
                blk = flat.reshape(ni // 16, 16).T      # [16, ni/16]
                idx16_a[c, :, col0:col0 + ni // 16] = np.tile(blk, (8, 1))

    xT_a = np.zeros((NC_, XD + 1, NB), np.float32)
    xT_a[:, XD, :] = 1.0
    bloc_a = np.full((NC_, NB), -1.0, np.float32)
    first_g = np.zeros(NC_, np.int64)
    for c in range(NC_):
        lo, hi = c * NB, min((c + 1) * NB, N)
        xT_a[c, :XD, : hi - lo] = x[lo:hi].T
        first_g[c] = batch[lo]
        assert batch[hi - 1] - batch[lo] < 128, "graph window exceeds 128"
        bloc_a[c, : hi - lo] = batch[lo:hi].astype(np.float32)

    prow_a = (first_g[:, None] + np.arange(128)[None, :]).astype(np.int32)
    assert prow_a.max() < cfg.PG
    cnt_g = np.bincount(batch, minlength=G).astype(np.float32)
    cnt_inv = np.zeros(cfg.PG, np.float32)
    cnt_inv[:G] = np.float32(1.0) / np.maximum(cnt_g, np.float32(1.0))

    def f32(a):
        return np.ascontiguousarray(np.asarray(a, np.float32))

    shared = dict(
        W_node=f32(inputs["W_node"]), b_node=f32(inputs["b_node"]).reshape(1, C),
        W_edge=f32(inputs["W_edge"]), b_edge=f32(inputs["b_edge"]).reshape(1, C),
        bnn_g=f32(inputs["bnn_g"]).reshape(1, C), bnn_b=f32(inputs["bnn_b"]).reshape(1, C),
        bnn_m=f32(inputs["bnn_m"]).reshape(1, C), bnn_v=f32(inputs["bnn_v"]).reshape(1, C),
        bne_g=f32(inputs["bne_g"]).reshape(1, C), bne_b=f32(inputs["bne_b"]).reshape(1, C),
        bne_m=f32(inputs["bne_m"]).reshape(1, C), bne_v=f32(inputs["bne_v"]).reshape(1, C),
        t=f32(inputs["t"]).reshape(1, L),
        W1=f32(inputs["W1"]), b1=f32(inputs["b1"]),
        bn1_g=f32(inputs["bn1_g"]), bn1_b=f32(inputs["bn1_b"]),
        bn1_m=f32(inputs["bn1_m"]), bn1_v=f32(inputs["bn1_v"]),
        W2=f32(inputs["W2"]), b2=f32(inputs["b2"]),
        W_out=f32(inputs["W_out"]), b_out=f32(inputs["b_out"]).reshape(1, 1),
        cnt_inv=cnt_inv,
    )
    in_maps = []
    for c in range(NC_):
        im = dict(shared)
        im.update(
            xT=xT_a[c], eaT=eaT_a[c].reshape(-1), gidx16=idx16_a[c],
            dloc=dloc_a[c], bloc=bloc_a[c], prow=prow_a[c],
        )
        in_maps.append(im)
    return in_maps


# ----------------------------------------------------------------------------
# Device program.
# ----------------------------------------------------------------------------

def emit_q(nc, ap, pre_bias_ap=None, clip=True):
    """In-place fake quantization of `ap` (fp32): q(x) (+fused bias if given)."""
    if pre_bias_ap is None:
        nc.scalar.activation(ap, ap, ACTF.Copy, bias=MAGIC, scale=QS)
    else:
        nc.scalar.activation(ap, ap, ACTF.Identity, bias=pre_bias_ap, scale=QS)
    nc.scalar.activation(ap, ap, ACTF.Copy, bias=QB2, scale=QI)
    if clip:
        nc.vector.tensor_scalar(ap, ap, QMAX, QMIN, AL.min, AL.max)


def build(cfg):
    C, L, TPC, NB = cfg.C, cfg.L, cfg.TPC, cfg.NB
    XD, ED, G, PG = cfg.XD, cfg.ED, cfg.G, cfg.PG
    NPAD, NR, RSZ = cfg.NPAD, cfg.NR, cfg.RSZ
    SLOTS, SCOLS, NCHG = cfg.SLOTS, cfg.SCOLS, cfg.NCHG
    C2 = 2 * C
    CE = 64                                          # padded h row (fp32)
    RG = [list(range(cfg.ncores))]
    SHARED = "Shared" if (cfg.use_shared and cfg.use_collectives) else "Local"

    nc = bacc.Bacc("TRN2", target_bir_lowering=False, debug=False,
                   enable_asserts=False, num_devices=cfg.ncores)

    # ---- kernel I/O ----
    d_xT = nc.dram_tensor("xT", [XD + 1, NB], F32, kind="ExternalInput")
    d_eaT = nc.dram_tensor("eaT", [(ED + 1) * SLOTS], F32, kind="ExternalInput")
    d_gidx = nc.dram_tensor("gidx16", [128, SCOLS], I16, kind="ExternalInput")
    d_dloc = nc.dram_tensor("dloc", [SLOTS], F32, kind="ExternalInput")
    d_bloc = nc.dram_tensor("bloc", [NB], F32, kind="ExternalInput")
    d_prow = nc.dram_tensor("prow", [128], I32, kind="ExternalInput")
    d_cntinv = nc.dram_tensor("cnt_inv", [PG], F32, kind="ExternalInput")
    d_Wn = nc.dram_tensor("W_node", [XD, C], F32, kind="ExternalInput")
    d_bn_ = nc.dram_tensor("b_node", [1, C], F32, kind="ExternalInput")
    d_We = nc.dram_tensor("W_edge", [ED, C], F32, kind="ExternalInput")
    d_be = nc.dram_tensor("b_edge", [1, C], F32, kind="ExternalInput")
    d_bnr = {k: nc.dram_tensor(k, [1, C], F32, kind="ExternalInput")
             for k in ["bnn_g", "bnn_b", "bnn_m", "bnn_v",
                       "bne_g", "bne_b", "bne_m", "bne_v"]}
    d_t = nc.dram_tensor("t", [1, L], F32, kind="ExternalInput")
    d_W1 = nc.dram_tensor("W1", [L, C, C2], F32, kind="ExternalInput")
    d_b1 = nc.dram_tensor("b1", [L, C2], F32, kind="ExternalInput")
    d_bn1 = {k: nc.dram_tensor(k, [L, C2], F32, kind="ExternalInput")
             for k in ["bn1_g", "bn1_b", "bn1_m", "bn1_v"]}
    d_W2 = nc.dram_tensor("W2", [L, C2, C], F32, kind="ExternalInput")
    d_b2 = nc.dram_tensor("b2", [L, C], F32, kind="ExternalInput")
    d_Wo = nc.dram_tensor("W_out", [C, 1], F32, kind="ExternalInput")
    d_bo = nc.dram_tensor("b_out", [1, 1], F32, kind="ExternalInput")
    d_out = nc.dram_tensor("out", [G, 1], F32, kind="ExternalOutput")
    d_hdbg = nc.dram_tensor("h_dbg", [NPAD, C], F32, kind="ExternalOutput")

    # ---- inline constants ----
    eye = np.eye(128, dtype=np.float32)
    iota4_np = np.tile(np.arange(128, dtype=np.float32), (128, 4, 1))
    ones_np = np.ones((1, 128), np.float32)
    c_eye = nc.inline_tensor(eye, "c_eye")
    c_iota4 = nc.inline_tensor(iota4_np, "c_iota4")
    NW = PG // 128                                   # pooling windows
    iota5_np = (np.tile(np.arange(128, dtype=np.float32), (128, NW, 1))
                + (np.arange(NW, dtype=np.float32) * 128)[None, :, None])
    c_iota5 = nc.inline_tensor(iota5_np, "c_iota5")
    c_ones = nc.inline_tensor(ones_np, "c_ones")

    with tile.TileContext(nc) as tc:
        with (
            tc.tile_pool(name="dram", bufs=1, space="DRAM") as dpool,
            tc.tile_pool(name="const", bufs=1) as cp,
        ):
            # ---- internal DRAM ----
            h64A = dpool.tile([NB, CE], F32, name="h64A")
            h64B = dpool.tile([NB, CE], F32, name="h64B")
            h64f = [dpool.tile([NPAD, CE], F32, addr_space=SHARED,
                               name=f"h64f_{l}") for l in range(L)]
            e_dram = dpool.tile([SLOTS * C], BF16, name="e_dram")
            xq_dram = dpool.tile([(XD + 1) * NB], F32, name="xq_dram")
            pool_glob = dpool.tile([PG, C], F32, name="pool_glob")
            pool_red = dpool.tile([PG, C], F32, addr_space=SHARED, name="pool_red")

            # ---- constants to SBUF ----
            nc.gpsimd.load_library(library_config.mlp)
            ident = cp.tile([128, 128], F32, name="ident")
            nc.sync.dma_start(ident[:, :], c_eye[:, :])
            iota4 = cp.tile([128, 4, 128], F32, name="iota4")
            nc.sync.dma_start(iota4[:, :, :], c_iota4[:, :, :])
            iota5 = cp.tile([128, NW, 128], F32, name="iota5")
            nc.sync.dma_start(iota5[:, :, :], c_iota5[:, :, :])
            pacc = cp.tile([128, NW, C], F32, name="pacc")
            nc.vector.memset(pacc[:, :, :], 0.0)
            onesr = cp.tile([1, 128], F32, name="onesr")
            nc.sync.dma_start(onesr[:, :], c_ones[:, :])

            # zero-fill h64 local buffers once (pad columns stay 0 forever)
            zt = cp.tile([128, 8, CE], F32, name="zt")
            nc.vector.memset(zt[:, :, :], 0.0)
            for hb in (h64A, h64B):
                for b in range(0, TPC, 8):
                    gs_ = min(8, TPC - b)
                    nc.sync.dma_start(
                        hb[b * 128:(b + gs_) * 128, :]
                        .rearrange("(t p) c -> p t c", p=128),
                        zt[:, :gs_, :])

            # ---- parameter prep ----
            rhs_node = cp.tile([XD + 1, C], F32, name="rhs_node")
            nc.sync.dma_start(rhs_node[:XD, :], d_Wn[:, :])
            nc.sync.dma_start(rhs_node[XD:XD + 1, :], d_bn_[:, :])
            emit_q(nc, rhs_node[:, :])
            rhs_edge = cp.tile([ED + 1, C], F32, name="rhs_edge")
            nc.sync.dma_start(rhs_edge[:ED, :], d_We[:, :])
            nc.sync.dma_start(rhs_edge[ED:ED + 1, :], d_be[:, :])
            emit_q(nc, rhs_edge[:, :])

            def bn_rows2(pref):
                g_ = cp.tile([1, C], F32, name=pref + "_g")
                b_ = cp.tile([1, C], F32, name=pref + "_b")
                m_ = cp.tile([1, C], F32, name=pref + "_m")
                sc = cp.tile([1, C], F32, name=pref + "_sc")
                bi = cp.tile([1, C], F32, name=pref + "_bi")
                tmp = cp.tile([1, C], F32, name=pref + "_tmp")
                nc.sync.dma_start(g_[:, :], d_bnr[pref + "_g"][:, :])
                nc.sync.dma_start(b_[:, :], d_bnr[pref + "_b"][:, :])
                nc.sync.dma_start(m_[:, :], d_bnr[pref + "_m"][:, :])
                nc.sync.dma_start(sc[:, :], d_bnr[pref + "_v"][:, :])
                nc.vector.tensor_scalar(sc[:, :], sc[:, :], BN_EPS, None, AL.add)
                nc.scalar.activation(sc[:, :], sc[:, :], ACTF.Sqrt)
                nc.vector.reciprocal(sc[:, :], sc[:, :])
                nc.vector.tensor_tensor(sc[:, :], sc[:, :], g_[:, :], op=AL.mult)
                nc.vector.tensor_tensor(bi[:, :], m_[:, :], sc[:, :], op=AL.mult)
                nc.vector.tensor_tensor(bi[:, :], b_[:, :], bi[:, :], op=AL.subtract)
                # fold q second step into BN: y = 1024*q+MAGIC
                # bn(q) = q*sc + bi = y*(sc/1024) + (bi - 12288*sc)
                nc.vector.tensor_scalar(tmp[:, :], sc[:, :], -12288.0, None, AL.mult)
                nc.vector.tensor_tensor(bi[:, :], bi[:, :], tmp[:, :], op=AL.add)
                nc.vector.tensor_scalar(sc[:, :], sc[:, :], QI, None, AL.mult)
                return sc, bi

            scN, biN = bn_rows2("bnn")
            scE, biE = bn_rows2("bne")

            def replicate4(row, nm, pool):
                ps = pool.tile([128, C], F32, name="rep_ps", tag="encp")
                nc.tensor.matmul(ps[:, :], lhsT=onesr[:, :], rhs=row[:, :],
                                 start=True, stop=True)
                out4 = cp.tile([128, 4 * C], F32, name=nm)
                for q in range(4):
                    nc.vector.tensor_copy(out4[:, q * C:(q + 1) * C], ps[:, :])
                return out4

            W1q, bias1, sc1, bi1, W2q, bias2 = [], [], [], [], [], []
            for l in range(L):
                w1 = cp.tile([C, C2], F32, name=f"W1q_{l}")
                nc.sync.dma_start(w1[:, :], d_W1[l, :, :])
                emit_q(nc, w1[:, :])
                W1q.append(w1)
                b1t = cp.tile([C2, 1], F32, name=f"bias1_{l}")
                nc.sync.dma_start(b1t[:, :], d_b1[l:l + 1, :].rearrange("a b -> b a"))
                emit_q(nc, b1t[:, :])
                nc.vector.tensor_scalar(b1t[:, :], b1t[:, :], QS, MAGIC, AL.mult, AL.add)
                bias1.append(b1t)

                g1 = cp.tile([C2, 1], F32, name=f"g1_{l}")
                bb1 = cp.tile([C2, 1], F32, name=f"bb1_{l}")
                m1 = cp.tile([C2, 1], F32, name=f"m1_{l}")
                s1 = cp.tile([C2, 1], F32, name=f"sc1_{l}")
                i1 = cp.tile([C2, 1], F32, name=f"bi1_{l}")
                nc.sync.dma_start(g1[:, :], d_bn1["bn1_g"][l:l + 1, :].rearrange("a b -> b a"))
                nc.sync.dma_start(bb1[:, :], d_bn1["bn1_b"][l:l + 1, :].rearrange("a b -> b a"))
                nc.sync.dma_start(m1[:, :], d_bn1["bn1_m"][l:l + 1, :].rearrange("a b -> b a"))
                nc.sync.dma_start(s1[:, :], d_bn1["bn1_v"][l:l + 1, :].rearrange("a b -> b a"))
                nc.vector.tensor_scalar(s1[:, :], s1[:, :], BN_EPS, None, AL.add)
                nc.scalar.activation(s1[:, :], s1[:, :], ACTF.Sqrt)
                nc.vector.reciprocal(s1[:, :], s1[:, :])
                nc.vector.tensor_tensor(s1[:, :], s1[:, :], g1[:, :], op=AL.mult)
                nc.vector.tensor_tensor(i1[:, :], m1[:, :], s1[:, :], op=AL.mult)
                nc.vector.tensor_tensor(i1[:, :], bb1[:, :], i1[:, :], op=AL.subtract)
                sc1.append(s1)
                bi1.append(i1)

                w2 = cp.tile([C2, C], F32, name=f"W2q_{l}")
                nc.sync.dma_start(w2[:, :], d_W2[l, :, :])
                emit_q(nc, w2[:, :])
                W2q.append(w2)
                b2t = cp.tile([C, 1], F32, name=f"bias2_{l}")
                nc.sync.dma_start(b2t[:, :], d_b2[l:l + 1, :].rearrange("a b -> b a"))
                emit_q(nc, b2t[:, :])
                nc.vector.tensor_scalar(b2t[:, :], b2t[:, :], QS, MAGIC, AL.mult, AL.add)
                bias2.append(b2t)

            Woq = cp.tile([C, 1], F32, name="Woq")
            nc.sync.dma_start(Woq[:, :], d_Wo[:, :])
            emit_q(nc, Woq[:, :])
            biaso = cp.tile([1, 1], F32, name="biaso")
            nc.sync.dma_start(biaso[:, :], d_bo[:, :])
            emit_q(nc, biaso[:, :])
            nc.vector.tensor_scalar(biaso[:, :], biaso[:, :], QS, MAGIC, AL.mult, AL.add)

            # ---- encoders ----
            def q_pass(src_flat, dst_flat, total, pool):
                per = total // 128
                assert total % 128 == 0
                W = min(per, 4096)
                n = (per + W - 1) // W
                sv = src_flat.rearrange("(p q) -> p q", p=128)
                dv = dst_flat.rearrange("(p q) -> p q", p=128)
                for i in range(n):
                    w = min(W, per - i * W)
                    tl = pool.tile([128, W], F32, tag="qpass", name="qpass")
                    nc.sync.dma_start(tl[:, :w], sv[:, i * W:i * W + w])
                    nc.scalar.activation(tl[:, :w], tl[:, :w], ACTF.Copy,
                                         bias=MAGIC, scale=QS)
                    nc.scalar.activation(tl[:, :w], tl[:, :w], ACTF.Copy,
                                         bias=QB2, scale=QI)
                    nc.sync.dma_start(dv[:, i * W:i * W + w], tl[:, :w])

            with (
                tc.tile_pool(name="enc", bufs=2) as enc,
                tc.tile_pool(name="encx", bufs=1) as encx,
                tc.tile_pool(name="encps", bufs=2, space="PSUM") as enc_ps,
            ):
                scN4 = replicate4(scN, "scN4", enc_ps)
                biN4 = replicate4(biN, "biN4", enc_ps)
                scE4 = replicate4(scE, "scE4", enc_ps)
                biE4 = replicate4(biE, "biE4", enc_ps)

                t_sb = cp.tile([1, L], F32, name="t_sb")
                nc.sync.dma_start(t_sb[:, :], d_t[:, :])
                t_ps = enc_ps.tile([128, L], F32, name="t_ps", tag="encp")
                nc.tensor.matmul(t_ps[:, :], lhsT=onesr[:, :], rhs=t_sb[:, :],
                                 start=True, stop=True)
                t_bc = cp.tile([128, L], F32, name="t_bc")
                nc.vector.tensor_copy(t_bc[:, :], t_ps[:, :])

                # node encoder (writes h64A cols 0:32, y-domain clip + foldedBN)
                q_pass(d_xT[:, :].rearrange("a b -> (a b)"), xq_dram[:],
                       (XD + 1) * NB, enc)
                xseg = encx.tile([XD + 1, NB], F32, name="xseg")
                nc.sync.dma_start(
                    xseg[:, :], xq_dram[:].rearrange("(r e) -> r e", r=XD + 1))
                for b in range(0, TPC, 4):
                    gs = min(4, TPC - b)
                    ep = enc_ps.tile([128, 4 * C], F32, name="encp", tag="encp")
                    for q in range(gs):
                        nc.tensor.matmul(
                            ep[:, q * C:(q + 1) * C],
                            lhsT=xseg[:, (b + q) * 128:(b + q + 1) * 128],
                            rhs=rhs_node[:, :], start=True, stop=True)
                    es = enc.tile([128, 4 * C], F32, name="encs", tag="encs")
                    # y = 1024*z + MAGIC (RNE snap)
                    nc.scalar.activation(es[:, :gs * C], ep[:, :gs * C], ACTF.Copy,
                                         bias=MAGIC, scale=QS)
                    nc.vector.tensor_scalar(es[:, :gs * C], es[:, :gs * C],
                                            YMAX, YMIN, AL.min, AL.max)
                    nc.vector.tensor_tensor(es[:, :gs * C], es[:, :gs * C],
                                            scN4[:, :gs * C], op=AL.mult)
                    nc.vector.tensor_tensor(es[:, :gs * C], es[:, :gs * C],
                                            biN4[:, :gs * C], op=AL.add)
                    nc.sync.dma_start(
                        h64A[b * 128:(b + gs) * 128, 0:C]
                        .rearrange("(t p) c -> p t c", p=128),
                        es[:, :gs * C].rearrange("p (t c) -> p t c", c=C))

                # edge encoder: quantize in SBUF, matmul, foldedBN, write bf16
                eav = d_eaT[:].rearrange("(r e) -> r e", r=ED + 1)
                n_ch = SLOTS // 128
                SEGC = 32
                for s0 in range(0, n_ch, SEGC):
                    sc_ = min(SEGC, n_ch - s0)
                    eseg = enc.tile([ED + 1, SEGC * 128], F32, name="eseg",
                                    tag="eseg")
                    nc.sync.dma_start(eseg[:, :sc_ * 128],
                                      eav[:, s0 * 128:(s0 + sc_) * 128])
                    nc.scalar.activation(eseg[:, :sc_ * 128], eseg[:, :sc_ * 128],
                                         ACTF.Copy, bias=MAGIC, scale=QS)
                    nc.scalar.activation(eseg[:, :sc_ * 128], eseg[:, :sc_ * 128],
                                         ACTF.Copy, bias=QB2, scale=QI)
                    for b in range(0, sc_, 4):
                        gs = min(4, sc_ - b)
                        ep = enc_ps.tile([128, 4 * C], F32, name="encp", tag="encp")
                        for q in range(gs):
                            nc.tensor.matmul(
                                ep[:, q * C:(q + 1) * C],
                                lhsT=eseg[:, (b + q) * 128:(b + q + 1) * 128],
                                rhs=rhs_edge[:, :], start=True, stop=True)
                        es = enc.tile([128, 4 * C], F32, name="encs2", tag="encs")
                        nc.scalar.activation(es[:, :gs * C], ep[:, :gs * C],
                                             ACTF.Copy, bias=MAGIC, scale=QS)
                        nc.vector.tensor_scalar(es[:, :gs * C], es[:, :gs * C],
                                                YMAX, YMIN, AL.min, AL.max)
                        nc.vector.tensor_tensor(es[:, :gs * C], es[:, :gs * C],
                                                scE4[:, :gs * C], op=AL.mult)
                        esb = enc.tile([128, 4 * C], BF16, name="esb", tag="esb")
                        nc.vector.tensor_tensor(esb[:, :gs * C], es[:, :gs * C],
                                                biE4[:, :gs * C], op=AL.add)
                        r0 = (s0 + b) * 128
                        nc.sync.dma_start(
                            e_dram[r0 * C:(r0 + gs * 128) * C]
                            .rearrange("(t p c) -> p t c", p=128, c=C),
                            esb[:, :gs * C].rearrange("p (t c) -> p t c", c=C))

                # first AllGather
                if cfg.use_collectives:
                    nc.gpsimd.collective_compute(
                        "AllGather", AL.bypass, replica_groups=RG,
                        ins=[h64A[:, :]], outs=[h64f[0][:, :]])
                else:
                    for b_ in range(cfg.ncores):
                        nc.sync.dma_start(h64f[0][b_ * NB:(b_ + 1) * NB, :],
                                          h64A[:, :])

            # ---- layers ----
            with (
                tc.tile_pool(name="edge", bufs=3) as epool,
                tc.tile_pool(name="hsgp", bufs=3) as hsgp,
                tc.tile_pool(name="node", bufs=2) as npool,
                tc.tile_pool(name="eps", bufs=1, space="PSUM") as ps_edge,
                tc.tile_pool(name="mlp1", bufs=1, space="PSUM") as ps_z1,
                tc.tile_pool(name="mlp2", bufs=1, space="PSUM") as ps_z2,
                tc.tile_pool(name="tr", bufs=1, space="PSUM") as ps_tr,
                tc.tile_pool(name="poolps", bufs=1, space="PSUM") as ps_pool,
            ):
              for l in range(min(L, cfg.n_layers)):
                  h_in = h64A if l % 2 == 0 else h64B
                  h_out = h64B if l % 2 == 0 else h64A
                  last = l == L - 1

                  for gi, grp in enumerate(cfg.groups):
                      F = cfg.group_F[gi]
                      cmeta = cfg.chunk_meta[gi]
                      imeta = cfg.instr_meta[gi]
                      ngt = len(grp)
                      c0 = imeta[0][3]
                      c1 = imeta[-1][3] + imeta[-1][1] * 8

                      idxt = epool.tile([128, cfg.MAXCOLS], I16, name="idxt",
                                        tag="idxt", padded_shape=[128, cfg.MAXCOLS])
                      nc.sync.dma_start(idxt[:, :c1 - c0], d_gidx[:, c0:c1])
                      eps_t = [ps_edge.tile([128, C2], F32, name=f"eps{t}",
                                            tag=f"eps{t}") for t in range(ngt)]

                      # --- segment-pipelined edge phase (one gather instr each) ---
                      for (ch0, nchi, r, col0) in imeta:
                          ni = nchi * 128
                          hsg = hsgp.tile([128, GMAXCH, CE], F32, name="hsg",
                                          tag="hsg")
                          nc.gpsimd.dma_gather(
                              hsg[:, :nchi, :],
                              h64f[l][r * RSZ:(r + 1) * RSZ, :],
                              idxt[:, col0 - c0:col0 - c0 + nchi * 8],
                              ni, ni, CE)
                          et = epool.tile([128, GMAXCH, C], BF16, name="et",
                                          tag="et")
                          nc.sync.dma_start(
                              et[:, :nchi, :],
                              e_dram[(F + ch0) * 128 * C:(F + ch0 + nchi) * 128 * C]
                              .rearrange("(t p c) -> p t c", p=128, c=C))
                          dlt = epool.tile([128, GMAXCH], F32, name="dlt",
                                           tag="dlt")
                          nc.sync.dma_start(
                              dlt[:, :nchi],
                              d_dloc[(F + ch0) * 128:(F + ch0 + nchi) * 128]
                              .rearrange("(k p) -> p k", p=128))
                          xb = epool.tile([128, GMAXCH, C], BF16, name="xb",
                                          tag="xb")
                          nc.vector.tensor_tensor(xb[:, :nchi, :],
                                                  hsg[:, :nchi, 0:C],
                                                  et[:, :nchi, :], op=AL.add)
                          exm = epool.tile([128, GMAXCH, C2], BF16, name="exm",
                                           tag="exm")
                          # den-term: gexp = exp(t*x); later max(gexp, 1)
                          nc.scalar.activation(exm[:, :nchi, C:C2],
                                               xb[:, :nchi, :], ACTF.Exp,
                                               scale=t_bc[:, l:l + 1])
                          # num-term: relu(x) * gexp (one fused op)
                          nc.vector.scalar_tensor_tensor(
                              exm[:, :nchi, 0:C], xb[:, :nchi, :], 0.0,
                              exm[:, :nchi, C:C2], op0=AL.max, op1=AL.mult)
                          nc.vector.tensor_scalar(exm[:, :nchi, C:C2],
                                                  exm[:, :nchi, C:C2],
                                                  1.0, None, AL.max)
                          oh = epool.tile([128, GMAXCH, 128], BF16, name="oh",
                                          tag="oh")
                          nc.vector.tensor_tensor(
                              oh[:, :nchi, :],
                              dlt[:, :nchi].to_broadcast([128, nchi, 128]),
                              iota4[:, 0:1, :].to_broadcast([128, nchi, 128]),
                              op=AL.is_equal)
                          for q in range(nchi):
                              t_, st, sp = cmeta[ch0 + q]
                              nc.tensor.matmul(
                                  eps_t[t_][:, :], lhsT=oh[:, q, :],
                                  rhs=exm[:, q, :], start=st, stop=sp)

                      # --- node phase per tile; MLP over the group ---
                      hog = None
                      h2qT = None
                      for ti, g in enumerate(grp):
                          tq = ti % 4
                          if tq == 0:
                              gs = min(4, ngt - ti)
                              hog = npool.tile([128, 4, C], F32, name="hog",
                                               tag="hog")
                              h2qT = npool.tile([C, 512], F32, name="h2qT",
                                                tag="h2qT")
                          nc.sync.dma_start(hog[:, tq, :],
                                            h_in[g * 128:(g + 1) * 128, 0:C])
                          dinv = npool.tile([128, C], F32, name="dinv", tag="dinv")
                          nc.vector.tensor_scalar(dinv[:, :], eps_t[ti][:, C:C2],
                                                  1e-16, None, AL.max)
                          nc.vector.reciprocal(dinv[:, :], dinv[:, :])
                          h2 = npool.tile([128, C], F32, name="h2", tag="h2")
                          nc.vector.tensor_tensor(h2[:, :], eps_t[ti][:, 0:C],
                                                  dinv[:, :], op=AL.mult)
                          nc.vector.tensor_tensor(h2[:, :], h2[:, :],
                                                  hog[:, tq, :], op=AL.add)
                          emit_q(nc, h2[:, :])
                          trp = ps_tr.tile([C, 128], F32, name="trp", tag="tr")
                          nc.tensor.transpose(trp[:, :], h2[:, :],
                                              identity=ident[:, :])
                          nc.vector.tensor_copy(h2qT[:, tq * 128:(tq + 1) * 128],
                                                trp[:, :])

                          if tq == gs - 1:
                              w = gs * 128
                              g0 = g - gs + 1
                              z1p = ps_z1.tile([C2, 512], F32, name="z1p", tag="z1p")
                              nc.tensor.matmul(z1p[:, :w], lhsT=W1q[l][:, :],
                                               rhs=h2qT[:, :w], start=True, stop=True)
                              z1s = npool.tile([C2, 512], F32, name="z1s", tag="z1s")
                              nc.scalar.activation(z1s[:, :w], z1p[:, :w],
                                                   ACTF.Identity,
                                                   bias=bias1[l][:, :], scale=QS)
                              nc.scalar.activation(z1s[:, :w], z1s[:, :w], ACTF.Copy,
                                                   bias=QB2, scale=QI)
                              nc.vector.tensor_scalar(z1s[:, :w], z1s[:, :w],
                                                      QMAX, QMIN, AL.min, AL.max)
                              nc.scalar.activation(z1s[:, :w], z1s[:, :w], ACTF.Relu,
                                                   bias=bi1[l][:, :], scale=sc1[l][:, :])
                              nc.scalar.activation(z1s[:, :w], z1s[:, :w], ACTF.Copy,
                                                   bias=MAGIC, scale=QS)
                              nc.scalar.activation(z1s[:, :w], z1s[:, :w], ACTF.Copy,
                                                   bias=QB2, scale=QI)
                              nc.vector.tensor_scalar(z1s[:, :w], z1s[:, :w],
                                                      QMAX, QMIN, AL.min, AL.max)
                              z2p = ps_z2.tile([C, 512], F32, name="z2p", tag="z2p")
                              nc.tensor.matmul(z2p[:, :w], lhsT=W2q[l][:, :],
                                               rhs=z1s[:, :w], start=True, stop=True)
                              z2s = npool.tile([C, 512], F32, name="z2s", tag="z2s")
                              nc.scalar.activation(z2s[:, :w], z2p[:, :w],
                                                   ACTF.Identity,
                                                   bias=bias2[l][:, :], scale=QS)
                              nc.scalar.activation(z2s[:, :w], z2s[:, :w], ACTF.Copy,
                                                   bias=QB2, scale=QI)
                              nc.vector.tensor_scalar(z2s[:, :w], z2s[:, :w],
                                                      QMAX, QMIN, AL.min, AL.max)
                              hnext = npool.tile([128, 4, C], F32, name="hnext",
                                                 tag="hnext")
                              for q in range(gs):
                                  trq = ps_tr.tile([128, C], F32, name="trq",
                                                   tag="tr")
                                  nc.tensor.transpose(trq[:, :],
                                                      z2s[:, q * 128:(q + 1) * 128],
                                                      identity=ident[0:C, 0:C])
                                  nc.vector.tensor_tensor(hnext[:, q, :], trq[:, :],
                                                          hog[:, q, :], op=AL.add)
                                  if last:
                                      blt = npool.tile([128, 1], F32, name="blt",
                                                       tag="blt")
                                      nc.sync.dma_start(
                                          blt[:, :],
                                          d_bloc[(g0 + q) * 128:(g0 + q + 1) * 128]
                                          .rearrange("(p one) -> p one", one=1))
                                      ohp = npool.tile([128, NW, 128], F32,
                                                       name="ohp", tag="ohp")
                                      nc.vector.tensor_tensor(
                                          ohp[:, :, :],
                                          blt[:, :].to_broadcast([128, NW, 128]),
                                          iota5[:, :, :], op=AL.is_equal)
                                      for wi in range(NW):
                                          pps = ps_pool.tile([128, C], F32,
                                                             name="pps", tag="pps")
                                          nc.tensor.matmul(
                                              pps[:, :], lhsT=ohp[:, wi, :],
                                              rhs=hnext[:, q, :],
                                              start=True, stop=True)
                                          nc.vector.tensor_tensor(
                                              pacc[:, wi, :], pacc[:, wi, :],
                                              pps[:, :], op=AL.add)
                              if not last:
                                  nc.sync.dma_start(
                                      h_out[g0 * 128:(g0 + gs) * 128, 0:C]
                                      .rearrange("(t p) c -> p t c", p=128),
                                      hnext[:, :gs, :])

                  if not last:
                      if cfg.use_collectives:
                          nc.gpsimd.collective_compute(
                              "AllGather", AL.bypass, replica_groups=RG,
                              ins=[h_out[:, :]], outs=[h64f[l + 1][:, :]])
                      else:
                          for b_ in range(cfg.ncores):
                              nc.sync.dma_start(
                                  h64f[l + 1][b_ * NB:(b_ + 1) * NB, :],
                                  h_out[:, :])

              if cfg.n_layers < L:
                  nl = cfg.n_layers
                  hf = h64f[min(nl, L - 1)]
                  for b_ in range(NPAD // 128):
                      dbg_t = npool.tile([128, C], F32, name="dbg_t", tag="dbg_t")
                      nc.sync.dma_start(dbg_t[:, :],
                                        hf[b_ * 128:(b_ + 1) * 128, 0:C])
                      nc.sync.dma_start(d_hdbg[b_ * 128:(b_ + 1) * 128, :],
                                        dbg_t[:, :])
                  return nc

              # ---- pooling: write window partials, AllReduce, output head ----
              nc.sync.dma_start(
                  pool_glob[:, :].rearrange("(w p) c -> p w c", p=128),
                  pacc[:, :, :])
              if cfg.use_collectives:
                  nc.gpsimd.collective_compute(
                      "AllReduce", AL.add, replica_groups=RG,
                      ins=[pool_glob[:, :]], outs=[pool_red[:, :]])
              else:
                  nc.sync.dma_start(pool_red[:, :], pool_glob[:, :])

              n_out_tiles = (G + 127) // 128
              for i in range(n_out_tiles):
                  w = min(128, G - i * 128)
                  pt = npool.tile([128, C], F32, name="pt", tag="pt")
                  nc.sync.dma_start(pt[:w, :], pool_red[i * 128:i * 128 + w, :])
                  civ = npool.tile([128, 1], F32, name="civ", tag="civ")
                  nc.sync.dma_start(civ[:w, :],
                                    d_cntinv[i * 128:i * 128 + w].rearrange("(p one) -> p one", one=1))
                  nc.vector.tensor_scalar(pt[:w, :], pt[:w, :], civ[:w, :], None, AL.mult)
                  emit_q(nc, pt[:w, :])
                  trh = ps_tr.tile([C, 128], F32, name="trh", tag="tr")
                  nc.tensor.transpose(trh[:, :w], pt[:w, :], identity=ident[:w, :w])
                  hts = npool.tile([C, 128], F32, name="hts", tag="hts")
                  nc.vector.tensor_copy(hts[:, :w], trh[:, :w])
                  op_ = ps_z2.tile([1, 128], F32, name="op_", tag="z2p")
                  nc.tensor.matmul(op_[:, :w], lhsT=Woq[:, :], rhs=hts[:, :w],
                                   start=True, stop=True)
                  osb = npool.tile([1, 128], F32, name="osb", tag="osb")
                  nc.scalar.activation(osb[:, :w], op_[:, :w], ACTF.Identity,
                                       bias=biaso[:, :], scale=QS)
                  nc.scalar.activation(osb[:, :w], osb[:, :w], ACTF.Copy,
                                       bias=QB2, scale=QI)
                  nc.vector.tensor_scalar(osb[:, :w], osb[:, :w], QMAX, QMIN,
                                          AL.min, AL.max)
                  nc.scalar.activation(osb[:, :w], osb[:, :w], ACTF.Sigmoid)
                  nc.scalar.activation(osb[:, :w], osb[:, :w], ACTF.Copy,
                                       bias=MAGIC, scale=QS)
                  nc.scalar.activation(osb[:, :w], osb[:, :w], ACTF.Copy,
                                       bias=QB2, scale=QI)
                  nc.sync.dma_start(
                      d_out[i * 128:i * 128 + w, :].rearrange("w one -> one w"),
                      osb[:, :w])

    return nc


# ----------------------------------------------------------------------------
# Entry point.
# ----------------------------------------------------------------------------

def run(inputs, cfg, **run_kwargs):
    global LAST_RESULTS
    in_maps = preprocess(inputs, cfg)
    nc = build(cfg)
    if not nc.is_finalized():
        nc.finalize()
    res = run_bass_kernel_spmd(nc, in_maps, core_ids=list(range(cfg.ncores)),
                               **run_kwargs)
    LAST_RESULTS = res
    return res.results[0]["out"].reshape(cfg.G, 1).astype(np.float32)


def kernel(**inputs) -> np.ndarray:
    cfg = Cfg(N=100000, E=3200000, G=512, XD=8, ED=4, C=32, L=4)
    return run(inputs, cfg)


# revision 28
# speedup vs baseline: 1.1827x; 1.0352x over previous
"""Trainium2 Bass kernel: nn_BV_Model (GENConv GNN, softmax aggregation, 4 layers).

Strategy (8 NeuronCores, SPMD), v2:
  - Nodes partitioned into 8 contiguous blocks (12544/core, padded); edges
    sorted by destination and bucketed per destination node-tile (128 nodes).
  - h is replicated per-core in DRAM as fp32 [NPAD, 64] (channels padded
    32->64 so one node row is 256B) and re-AllGathered per layer.
  - The per-edge h[src] gather uses the batched SWDGE dma_gather
    (InstDMAGatherAnt): 1024 edges per instruction (vs. one 128-edge chunk
    per indirect_dma_start), with edges grouped per (8-tile supertile,
    src-range) so int16 indices stay in range (4 ranges of NPAD/4 rows).
  - Edge math: x = h_src + e; with t>=0, exp(t*relu(x)) == max(exp(t*x), 1),
    so num-term relu(x)*ex is one fused scalar_tensor_tensor op and den-term
    is one tensor_scalar max. eps=1e-7 is dropped (error ~1e-7 << tol).
  - Segment softmax reduced edges->nodes with a one-hot(dst) matmul in bf16
    accumulated in PSUM; e / exmex / one-hot are bf16 (tol 2e-2).
  - Node MLP on the tensor engine in transposed layout (fp32, exact quant).
  - Global mean pool per-core with one-hot(graph) matmuls + AllReduce.

Fake-quantization q(x) = clip(rne(x*1024), -32768, 32767)/1024 via the
round-to-nearest-even "magic number" trick (+1.5*2^23).
"""

import os
os.environ.setdefault("MYCRO_LOCAL_CACHE", "1")

import math
import numpy as np

import concourse.bacc as bacc
import concourse.tile as tile
import concourse.bass as bass
from concourse import mybir
from concourse import library_config
from concourse.bass_utils import run_bass_kernel_spmd

F32 = mybir.dt.float32
BF16 = mybir.dt.bfloat16
I16 = mybir.dt.int16
I32 = mybir.dt.int32
ACTF = mybir.ActivationFunctionType
AL = mybir.AluOpType

MAGIC = 12582912.0           # 1.5*2^23 : fp32 RNE rounding magic
QS = 1024.0                  # 2^10
QI = 1.0 / 1024.0
QB2 = -12288.0               # -MAGIC * 2^-10
QMAX = 32767.0 / 1024.0
QMIN = -32.0
# clip bounds in the y = 1024*q + MAGIC domain (pre-descale)
YMAX = QMAX * QS + MAGIC
YMIN = QMIN * QS + MAGIC
BN_EPS = 1e-5
NCORES = 8
GT = 4                       # tiles per gather/edge supertile (= MLP group)
GMAXCH = 8                   # max chunks (of 128 idx) per dma_gather instr

LAST_RESULTS = None          # BassKernelResults of the most recent run


class Cfg:
    def __init__(self, N, E, G, XD=8, ED=4, C=32, L=4, ncores=NCORES,
                 use_collectives=True, use_shared=True, gather_k=None,
                 no_indirect=False, gather_plain=True, n_layers=None):
        self.N, self.E, self.G = N, E, G
        self.XD, self.ED, self.C, self.L = XD, ED, C, L
        self.ncores = ncores
        self.use_collectives = use_collectives and ncores > 1
        self.use_shared = use_shared
        self.n_layers = L if n_layers is None else n_layers
        self.TPC = (N + ncores * 128 - 1) // (ncores * 128)    # node tiles per core
        self.NB = self.TPC * 128                               # nodes per core (padded)
        self.NPAD = self.NB * ncores
        self.NR = 4
        assert self.NPAD % self.NR == 0
        self.RSZ = self.NPAD // self.NR                        # src range rows
        assert self.RSZ <= 32767
        self.PG = ((G + 128) + 127) // 128 * 128               # pooled scatter rows
        # filled by preprocess:
        self.groups = None          # list[list[g]]
        self.chunk_meta = None      # per group: list[(t_in_group, start, stop)]
        self.instr_meta = None      # per group: list[(ch0_local, nch, r, col0)]
        self.group_F = None         # per group: global chunk offset
        self.NCHG = None            # max chunks in a group
        self.SLOTS = None
        self.SCOLS = None


# ----------------------------------------------------------------------------
# Host-side preprocessing: sort/bucket edges, build per-core input arrays.
# ----------------------------------------------------------------------------

def preprocess(inputs, cfg):
    x = np.ascontiguousarray(np.asarray(inputs["x"], np.float32))
    ea = np.ascontiguousarray(np.asarray(inputs["edge_attr"], np.float32))
    ei = np.asarray(inputs["edge_index"]).astype(np.int64)
    batch = np.asarray(inputs["batch"]).astype(np.int64)
    N, E, G = cfg.N, cfg.E, cfg.G
    XD, ED, C, L = cfg.XD, cfg.ED, cfg.C, cfg.L
    TPC, NB, NR, RSZ = cfg.TPC, cfg.NB, cfg.NR, cfg.RSZ
    NC_ = cfg.ncores

    src, dst = ei[0], ei[1]
    order = np.argsort(dst, kind="stable")
    src_s = src[order]
    dst_s = dst[order]
    ea_s = ea[order]

    ntiles = NC_ * TPC
    bnd = np.searchsorted(dst_s, np.arange(ntiles + 1) * 128)

    # per (core, tile): reorder edges by src range; count per range
    cnt = np.zeros((NC_, TPC, NR), np.int64)
    seg = {}                       # (c, g, r) -> (src_rel, dloc, ea rows)
    for c in range(NC_):
        for g in range(TPC):
            t = c * TPC + g
            b0, b1 = int(bnd[t]), int(bnd[t + 1])
            if b1 <= b0:
                continue
            s = src_s[b0:b1]
            d = dst_s[b0:b1]
            e_ = ea_s[b0:b1]
            r = s // RSZ
            o = np.argsort(r, kind="stable")
            s, d, e_, r = s[o], d[o], e_[o], r[o]
            rb = np.searchsorted(r, np.arange(NR + 1))
            for rr in range(NR):
                m = int(rb[rr + 1] - rb[rr])
                cnt[c, g, rr] = m
                if m:
                    sl = slice(int(rb[rr]), int(rb[rr + 1]))
                    seg[(c, g, rr)] = (
                        (s[sl] - rr * RSZ).astype(np.int16),
                        (d[sl] - t * 128).astype(np.float32),
                        e_[sl],
                    )

    K = (cnt.max(axis=0) + 127) // 128                         # [TPC, NR]

    # group structure + chunk/instr tables (shared across cores)
    groups = [list(range(g0, min(g0 + GT, TPC))) for g0 in range(0, TPC, GT)]
    chunk_meta, instr_meta, group_F = [], [], []
    ch_start = {}                  # (g, r) -> global chunk index
    ch = 0
    for grp in groups:
        group_F.append(ch)
        cm = []
        im = []
        # first/last chunk index (local) per tile for start/stop flags
        tile_chunks = {ti: [] for ti in range(len(grp))}
        local = 0
        runs = []
        for r in range(NR):
            run0 = local
            for ti, g in enumerate(grp):
                for _ in range(int(K[g, r])):
                    cm.append([ti, False, False])
                    tile_chunks[ti].append(local)
                    local += 1
            runs.append((run0, local - run0, r))
        for ti, lst in tile_chunks.items():
            if lst:
                cm[lst[0]][1] = True
                cm[lst[-1]][2] = True
        for (run0, n, r) in runs:
            o = 0
            while o < n:
                nch = min(GMAXCH, n - o)
                im.append([run0 + o, nch, r, 0])
                o += nch
        chunk_meta.append([tuple(e) for e in cm])
        instr_meta.append(im)
        for r in range(NR):
            for g in grp:
                ch_start[(g, r)] = ch
                ch += int(K[g, r])
    NCH_TOT = ch
    SLOTS = NCH_TOT * 128
    # idx column offsets (global, shared)
    col = 0
    maxcols = 0
    for gi in range(len(groups)):
        col0g = col
        for e in instr_meta[gi]:
            e[3] = col
            col += e[1] * 8  # nch*128/16 = nch*8 columns
        maxcols = max(maxcols, col - col0g)
        instr_meta[gi] = [tuple(e) for e in instr_meta[gi]]
    SCOLS = col

    cfg.groups, cfg.chunk_meta, cfg.instr_meta = groups, chunk_meta, instr_meta
    cfg.group_F = group_F
    cfg.NCHG = max(len(cm) for cm in chunk_meta)
    cfg.MAXCOLS = maxcols
    cfg.SLOTS, cfg.SCOLS = SLOTS, SCOLS
    cfg.NCH_TOT = NCH_TOT

    # per-core slot arrays
    idxrel_a = np.zeros((NC_, SLOTS), np.int16)
    dloc_a = np.full((NC_, SLOTS), -1.0, np.float32)
    eaT_a = np.zeros((NC_, ED + 1, SLOTS), np.float32)
    eaT_a[:, ED, :] = 1.0
    for c in range(NC_):
        for g in range(TPC):
            for r in range(NR):
                m = int(cnt[c, g, r])
                if m == 0:
                    continue
                s0 = ch_start[(g, r)] * 128
                sr, dl, er = seg[(c, g, r)]
                idxrel_a[c, s0:s0 + m] = sr
                dloc_a[c, s0:s0 + m] = dl
                eaT_a[c, :ED, s0:s0 + m] = er.T
    # dloc in partition-major [128, NCH_TOT] layout (slot = ch*128 + p)
    dloc_pm = np.ascontiguousarray(
        dloc_a.reshape(NC_, NCH_TOT, 128).transpose(0, 2, 1))

    # wrapped int16 index arrays per instruction
    idx16_a = np.zeros((NC_, 128, SCOLS), np.int16)
    for gi, grp in enumerate(groups):
        F = group_F[gi]
        for (ch0, nch, r, col0) in instr_meta[gi]:
            ni = nch * 128
            s0 = (F + ch0) * 128
            for c in range(NC_):
                flat = idxrel_a[c, s0:s0 + ni]
                blk = flat.reshape(ni // 16, 16).T      # [16, ni/16]
                idx16_a[c, :, col0:col0 + ni // 16] = np.tile(blk, (8, 1))

    xT_a = np.zeros((NC_, XD + 1, NB), np.float32)
    xT_a[:, XD, :] = 1.0
    bloc_a = np.full((NC_, NB), -1.0, np.float32)
    first_g = np.zeros(NC_, np.int64)
    for c in range(NC_):
        lo, hi = c * NB, min((c + 1) * NB, N)
        xT_a[c, :XD, : hi - lo] = x[lo:hi].T
        first_g[c] = batch[lo]
        assert batch[hi - 1] - batch[lo] < 128, "graph window exceeds 128"
        bloc_a[c, : hi - lo] = batch[lo:hi].astype(np.float32)

    prow_a = (first_g[:, None] + np.arange(128)[None, :]).astype(np.int32)
    assert prow_a.max() < cfg.PG
    cnt_g = np.bincount(batch, minlength=G).astype(np.float32)
    cnt_inv = np.zeros(cfg.PG, np.float32)
    cnt_inv[:G] = np.float32(1.0) / np.maximum(cnt_g, np.float32(1.0))

    def f32(a):
        return np.ascontiguousarray(np.asarray(a, np.float32))

    shared = dict(
        W_node=f32(inputs["W_node"]), b_node=f32(inputs["b_node"]).reshape(1, C),
        W_edge=f32(inputs["W_edge"]), b_edge=f32(inputs["b_edge"]).reshape(1, C),
        bnn_g=f32(inputs["bnn_g"]).reshape(1, C), bnn_b=f32(inputs["bnn_b"]).reshape(1, C),
        bnn_m=f32(inputs["bnn_m"]).reshape(1, C), bnn_v=f32(inputs["bnn_v"]).reshape(1, C),
        bne_g=f32(inputs["bne_g"]).reshape(1, C), bne_b=f32(inputs["bne_b"]).reshape(1, C),
        bne_m=f32(inputs["bne_m"]).reshape(1, C), bne_v=f32(inputs["bne_v"]).reshape(1, C),
        t=f32(inputs["t"]).reshape(1, L),
        W1=f32(inputs["W1"]), b1=f32(inputs["b1"]),
        bn1_g=f32(inputs["bn1_g"]), bn1_b=f32(inputs["bn1_b"]),
        bn1_m=f32(inputs["bn1_m"]), bn1_v=f32(inputs["bn1_v"]),
        W2=f32(inputs["W2"]), b2=f32(inputs["b2"]),
        W_out=f32(inputs["W_out"]), b_out=f32(inputs["b_out"]).reshape(1, 1),
        cnt_inv=cnt_inv,
    )
    in_maps = []
    for c in range(NC_):
        im = dict(shared)
        im.update(
            xT=xT_a[c], eaT=eaT_a[c].reshape(-1), gidx16=idx16_a[c],
            dloc=dloc_pm[c], bloc=bloc_a[c], prow=prow_a[c],
        )
        in_maps.append(im)
    return in_maps


# ----------------------------------------------------------------------------
# Device program.
# ----------------------------------------------------------------------------

def emit_q(nc, ap, pre_bias_ap=None, clip=True):
    """In-place fake quantization of `ap` (fp32): q(x) (+fused bias if given)."""
    if pre_bias_ap is None:
        nc.scalar.activation(ap, ap, ACTF.Copy, bias=MAGIC, scale=QS)
    else:
        nc.scalar.activation(ap, ap, ACTF.Identity, bias=pre_bias_ap, scale=QS)
    nc.scalar.activation(ap, ap, ACTF.Copy, bias=QB2, scale=QI)
    if clip:
        nc.vector.tensor_scalar(ap, ap, QMAX, QMIN, AL.min, AL.max)


def build(cfg):
    C, L, TPC, NB = cfg.C, cfg.L, cfg.TPC, cfg.NB
    XD, ED, G, PG = cfg.XD, cfg.ED, cfg.G, cfg.PG
    NPAD, NR, RSZ = cfg.NPAD, cfg.NR, cfg.RSZ
    SLOTS, SCOLS, NCHG = cfg.SLOTS, cfg.SCOLS, cfg.NCHG
    C2 = 2 * C
    CE = 64                                          # padded h row (fp32)
    RG = [list(range(cfg.ncores))]
    SHARED = "Shared" if (cfg.use_shared and cfg.use_collectives) else "Local"

    nc = bacc.Bacc("TRN2", target_bir_lowering=False, debug=False,
                   enable_asserts=False, num_devices=cfg.ncores)

    # ---- kernel I/O ----
    NCH_TOT = cfg.NCH_TOT
    d_xT = nc.dram_tensor("xT", [XD + 1, NB], F32, kind="ExternalInput")
    d_eaT = nc.dram_tensor("eaT", [(ED + 1) * SLOTS], F32, kind="ExternalInput")
    d_gidx = nc.dram_tensor("gidx16", [128, SCOLS], I16, kind="ExternalInput")
    d_dloc = nc.dram_tensor("dloc", [128, NCH_TOT], F32, kind="ExternalInput")
    d_bloc = nc.dram_tensor("bloc", [NB], F32, kind="ExternalInput")
    d_prow = nc.dram_tensor("prow", [128], I32, kind="ExternalInput")
    d_cntinv = nc.dram_tensor("cnt_inv", [PG], F32, kind="ExternalInput")
    d_Wn = nc.dram_tensor("W_node", [XD, C], F32, kind="ExternalInput")
    d_bn_ = nc.dram_tensor("b_node", [1, C], F32, kind="ExternalInput")
    d_We = nc.dram_tensor("W_edge", [ED, C], F32, kind="ExternalInput")
    d_be = nc.dram_tensor("b_edge", [1, C], F32, kind="ExternalInput")
    d_bnr = {k: nc.dram_tensor(k, [1, C], F32, kind="ExternalInput")
             for k in ["bnn_g", "bnn_b", "bnn_m", "bnn_v",
                       "bne_g", "bne_b", "bne_m", "bne_v"]}
    d_t = nc.dram_tensor("t", [1, L], F32, kind="ExternalInput")
    d_W1 = nc.dram_tensor("W1", [L, C, C2], F32, kind="ExternalInput")
    d_b1 = nc.dram_tensor("b1", [L, C2], F32, kind="ExternalInput")
    d_bn1 = {k: nc.dram_tensor(k, [L, C2], F32, kind="ExternalInput")
             for k in ["bn1_g", "bn1_b", "bn1_m", "bn1_v"]}
    d_W2 = nc.dram_tensor("W2", [L, C2, C], F32, kind="ExternalInput")
    d_b2 = nc.dram_tensor("b2", [L, C], F32, kind="ExternalInput")
    d_Wo = nc.dram_tensor("W_out", [C, 1], F32, kind="ExternalInput")
    d_bo = nc.dram_tensor("b_out", [1, 1], F32, kind="ExternalInput")
    d_out = nc.dram_tensor("out", [G, 1], F32, kind="ExternalOutput")
    d_hdbg = nc.dram_tensor("h_dbg", [NPAD, C], F32, kind="ExternalOutput")

    # ---- inline constants ----
    eye = np.eye(128, dtype=np.float32)
    iota4_np = np.tile(np.arange(128, dtype=np.float32), (128, 4, 1))
    ones_np = np.ones((1, 128), np.float32)
    c_eye = nc.inline_tensor(eye, "c_eye")
    c_iota4 = nc.inline_tensor(iota4_np, "c_iota4")
    NW = PG // 128                                   # pooling windows
    iota5_np = (np.tile(np.arange(128, dtype=np.float32), (128, NW, 1))
                + (np.arange(NW, dtype=np.float32) * 128)[None, :, None])
    c_iota5 = nc.inline_tensor(iota5_np, "c_iota5")
    c_ones = nc.inline_tensor(ones_np, "c_ones")

    with tile.TileContext(nc) as tc:
        with (
            tc.tile_pool(name="dram", bufs=1, space="DRAM") as dpool,
            tc.tile_pool(name="const", bufs=1) as cp,
        ):
            # ---- internal DRAM ----
            h64A = dpool.tile([NB, CE], F32, name="h64A")
            h64B = dpool.tile([NB, CE], F32, name="h64B")
            h64f = [dpool.tile([NPAD, CE], F32, addr_space=SHARED,
                               name=f"h64f_{l}") for l in range(L)]
            e_dram = dpool.tile([128, NCH_TOT, C], BF16, name="e_dram")
            xq_dram = dpool.tile([(XD + 1) * NB], F32, name="xq_dram")
            pool_glob = dpool.tile([PG, C], F32, name="pool_glob")
            pool_red = dpool.tile([PG, C], F32, addr_space=SHARED, name="pool_red")

            # ---- constants to SBUF ----
            nc.gpsimd.load_library(library_config.mlp)
            ident = cp.tile([128, 128], F32, name="ident")
            nc.sync.dma_start(ident[:, :], c_eye[:, :])
            iota4 = cp.tile([128, 4, 128], F32, name="iota4")
            nc.sync.dma_start(iota4[:, :, :], c_iota4[:, :, :])
            iota5 = cp.tile([128, NW, 128], F32, name="iota5")
            nc.sync.dma_start(iota5[:, :, :], c_iota5[:, :, :])
            pacc = cp.tile([128, NW, C], F32, name="pacc")
            nc.vector.memset(pacc[:, :, :], 0.0)
            onesr = cp.tile([1, 128], F32, name="onesr")
            nc.sync.dma_start(onesr[:, :], c_ones[:, :])

            # zero-fill h64 local buffers once (pad columns stay 0 forever)
            zt = cp.tile([128, 8, CE], F32, name="zt")
            nc.vector.memset(zt[:, :, :], 0.0)
            for hb in (h64A, h64B):
                for b in range(0, TPC, 8):
                    gs_ = min(8, TPC - b)
                    nc.sync.dma_start(
                        hb[b * 128:(b + gs_) * 128, :]
                        .rearrange("(t p) c -> p t c", p=128),
                        zt[:, :gs_, :])

            # ---- parameter prep ----
            rhs_node = cp.tile([XD + 1, C], F32, name="rhs_node")
            nc.sync.dma_start(rhs_node[:XD, :], d_Wn[:, :])
            nc.sync.dma_start(rhs_node[XD:XD + 1, :], d_bn_[:, :])
            emit_q(nc, rhs_node[:, :])
            rhs_edge = cp.tile([ED + 1, C], F32, name="rhs_edge")
            nc.sync.dma_start(rhs_edge[:ED, :], d_We[:, :])
            nc.sync.dma_start(rhs_edge[ED:ED + 1, :], d_be[:, :])
            emit_q(nc, rhs_edge[:, :])

            def bn_rows2(pref):
                g_ = cp.tile([1, C], F32, name=pref + "_g")
                b_ = cp.tile([1, C], F32, name=pref + "_b")
                m_ = cp.tile([1, C], F32, name=pref + "_m")
                sc = cp.tile([1, C], F32, name=pref + "_sc")
                bi = cp.tile([1, C], F32, name=pref + "_bi")
                tmp = cp.tile([1, C], F32, name=pref + "_tmp")
                nc.sync.dma_start(g_[:, :], d_bnr[pref + "_g"][:, :])
                nc.sync.dma_start(b_[:, :], d_bnr[pref + "_b"][:, :])
                nc.sync.dma_start(m_[:, :], d_bnr[pref + "_m"][:, :])
                nc.sync.dma_start(sc[:, :], d_bnr[pref + "_v"][:, :])
                nc.vector.tensor_scalar(sc[:, :], sc[:, :], BN_EPS, None, AL.add)
                nc.scalar.activation(sc[:, :], sc[:, :], ACTF.Sqrt)
                nc.vector.reciprocal(sc[:, :], sc[:, :])
                nc.vector.tensor_tensor(sc[:, :], sc[:, :], g_[:, :], op=AL.mult)
                nc.vector.tensor_tensor(bi[:, :], m_[:, :], sc[:, :], op=AL.mult)
                nc.vector.tensor_tensor(bi[:, :], b_[:, :], bi[:, :], op=AL.subtract)
                # fold q second step into BN: y = 1024*q+MAGIC
                # bn(q) = q*sc + bi = y*(sc/1024) + (bi - 12288*sc)
                nc.vector.tensor_scalar(tmp[:, :], sc[:, :], -12288.0, None, AL.mult)
                nc.vector.tensor_tensor(bi[:, :], bi[:, :], tmp[:, :], op=AL.add)
                nc.vector.tensor_scalar(sc[:, :], sc[:, :], QI, None, AL.mult)
                return sc, bi

            scN, biN = bn_rows2("bnn")
            scE, biE = bn_rows2("bne")

            def replicate4(row, nm, pool):
                ps = pool.tile([128, C], F32, name="rep_ps", tag="encp")
                nc.tensor.matmul(ps[:, :], lhsT=onesr[:, :], rhs=row[:, :],
                                 start=True, stop=True)
                out4 = cp.tile([128, 4 * C], F32, name=nm)
                for q in range(4):
                    nc.vector.tensor_copy(out4[:, q * C:(q + 1) * C], ps[:, :])
                return out4

            W1q, bias1, sc1, bi1, W2q, bias2 = [], [], [], [], [], []
            for l in range(L):
                w1 = cp.tile([C, C2], F32, name=f"W1q_{l}")
                nc.sync.dma_start(w1[:, :], d_W1[l, :, :])
                emit_q(nc, w1[:, :])
                W1q.append(w1)
                b1t = cp.tile([C2, 1], F32, name=f"bias1_{l}")
                nc.sync.dma_start(b1t[:, :], d_b1[l:l + 1, :].rearrange("a b -> b a"))
                emit_q(nc, b1t[:, :])
                nc.vector.tensor_scalar(b1t[:, :], b1t[:, :], QS, MAGIC, AL.mult, AL.add)
                bias1.append(b1t)

                g1 = cp.tile([C2, 1], F32, name=f"g1_{l}")
                bb1 = cp.tile([C2, 1], F32, name=f"bb1_{l}")
                m1 = cp.tile([C2, 1], F32, name=f"m1_{l}")
                s1 = cp.tile([C2, 1], F32, name=f"sc1_{l}")
                i1 = cp.tile([C2, 1], F32, name=f"bi1_{l}")
                nc.sync.dma_start(g1[:, :], d_bn1["bn1_g"][l:l + 1, :].rearrange("a b -> b a"))
                nc.sync.dma_start(bb1[:, :], d_bn1["bn1_b"][l:l + 1, :].rearrange("a b -> b a"))
                nc.sync.dma_start(m1[:, :], d_bn1["bn1_m"][l:l + 1, :].rearrange("a b -> b a"))
                nc.sync.dma_start(s1[:, :], d_bn1["bn1_v"][l:l + 1, :].rearrange("a b -> b a"))
                nc.vector.tensor_scalar(s1[:, :], s1[:, :], BN_EPS, None, AL.add)
                nc.scalar.activation(s1[:, :], s1[:, :], ACTF.Sqrt)
                nc.vector.reciprocal(s1[:, :], s1[:, :])
                nc.vector.tensor_tensor(s1[:, :], s1[:, :], g1[:, :], op=AL.mult)
                nc.vector.tensor_tensor(i1[:, :], m1[:, :], s1[:, :], op=AL.mult)
                nc.vector.tensor_tensor(i1[:, :], bb1[:, :], i1[:, :], op=AL.subtract)
                sc1.append(s1)
                bi1.append(i1)

                w2 = cp.tile([C2, C], F32, name=f"W2q_{l}")
                nc.sync.dma_start(w2[:, :], d_W2[l, :, :])
                emit_q(nc, w2[:, :])
                W2q.append(w2)
                b2t = cp.tile([C, 1], F32, name=f"bias2_{l}")
                nc.sync.dma_start(b2t[:, :], d_b2[l:l + 1, :].rearrange("a b -> b a"))
                emit_q(nc, b2t[:, :])
                nc.vector.tensor_scalar(b2t[:, :], b2t[:, :], QS, MAGIC, AL.mult, AL.add)
                bias2.append(b2t)

            Woq = cp.tile([C, 1], F32, name="Woq")
            nc.sync.dma_start(Woq[:, :], d_Wo[:, :])
            emit_q(nc, Woq[:, :])
            biaso = cp.tile([1, 1], F32, name="biaso")
            nc.sync.dma_start(biaso[:, :], d_bo[:, :])
            emit_q(nc, biaso[:, :])
            nc.vector.tensor_scalar(biaso[:, :], biaso[:, :], QS, MAGIC, AL.mult, AL.add)

            # ---- encoders ----
            def q_pass(src_flat, dst_flat, total, pool):
                per = total // 128
                assert total % 128 == 0
                W = min(per, 4096)
                n = (per + W - 1) // W
                sv = src_flat.rearrange("(p q) -> p q", p=128)
                dv = dst_flat.rearrange("(p q) -> p q", p=128)
                for i in range(n):
                    w = min(W, per - i * W)
                    tl = pool.tile([128, W], F32, tag="qpass", name="qpass")
                    nc.sync.dma_start(tl[:, :w], sv[:, i * W:i * W + w])
                    nc.scalar.activation(tl[:, :w], tl[:, :w], ACTF.Copy,
                                         bias=MAGIC, scale=QS)
                    nc.scalar.activation(tl[:, :w], tl[:, :w], ACTF.Copy,
                                         bias=QB2, scale=QI)
                    nc.sync.dma_start(dv[:, i * W:i * W + w], tl[:, :w])

            with (
                tc.tile_pool(name="enc", bufs=2) as enc,
                tc.tile_pool(name="encx", bufs=1) as encx,
                tc.tile_pool(name="encps", bufs=2, space="PSUM") as enc_ps,
            ):
                scN4 = replicate4(scN, "scN4", enc_ps)
                biN4 = replicate4(biN, "biN4", enc_ps)
                scE4 = replicate4(scE, "scE4", enc_ps)
                biE4 = replicate4(biE, "biE4", enc_ps)

                t_sb = cp.tile([1, L], F32, name="t_sb")
                nc.sync.dma_start(t_sb[:, :], d_t[:, :])
                t_ps = enc_ps.tile([128, L], F32, name="t_ps", tag="encp")
                nc.tensor.matmul(t_ps[:, :], lhsT=onesr[:, :], rhs=t_sb[:, :],
                                 start=True, stop=True)
                t_bc = cp.tile([128, L], F32, name="t_bc")
                nc.vector.tensor_copy(t_bc[:, :], t_ps[:, :])

                # node encoder (writes h64A cols 0:32, y-domain clip + foldedBN)
                q_pass(d_xT[:, :].rearrange("a b -> (a b)"), xq_dram[:],
                       (XD + 1) * NB, enc)
                xseg = encx.tile([XD + 1, NB], F32, name="xseg")
                nc.sync.dma_start(
                    xseg[:, :], xq_dram[:].rearrange("(r e) -> r e", r=XD + 1))
                for b in range(0, TPC, 4):
                    gs = min(4, TPC - b)
                    ep = enc_ps.tile([128, 4 * C], F32, name="encp", tag="encp")
                    for q in range(gs):
                        nc.tensor.matmul(
                            ep[:, q * C:(q + 1) * C],
                            lhsT=xseg[:, (b + q) * 128:(b + q + 1) * 128],
                            rhs=rhs_node[:, :], start=True, stop=True)
                    es = enc.tile([128, 4 * C], F32, name="encs", tag="encs")
                    # y = 1024*z + MAGIC (RNE snap)
                    nc.scalar.activation(es[:, :gs * C], ep[:, :gs * C], ACTF.Copy,
                                         bias=MAGIC, scale=QS)
                    nc.vector.tensor_scalar(es[:, :gs * C], es[:, :gs * C],
                                            YMAX, YMIN, AL.min, AL.max)
                    nc.vector.tensor_tensor(es[:, :gs * C], es[:, :gs * C],
                                            scN4[:, :gs * C], op=AL.mult)
                    nc.vector.tensor_tensor(es[:, :gs * C], es[:, :gs * C],
                                            biN4[:, :gs * C], op=AL.add)
                    nc.sync.dma_start(
                        h64A[b * 128:(b + gs) * 128, 0:C]
                        .rearrange("(t p) c -> p t c", p=128),
                        es[:, :gs * C].rearrange("p (t c) -> p t c", c=C))

                # edge encoder: quantize in SBUF, matmul, foldedBN, write bf16
                eav = d_eaT[:].rearrange("(r e) -> r e", r=ED + 1)
                n_ch = SLOTS // 128
                SEGC = 32
                for s0 in range(0, n_ch, SEGC):
                    sc_ = min(SEGC, n_ch - s0)
                    eseg = enc.tile([ED + 1, SEGC * 128], F32, name="eseg",
                                    tag="eseg")
                    nc.sync.dma_start(eseg[:, :sc_ * 128],
                                      eav[:, s0 * 128:(s0 + sc_) * 128])
                    nc.scalar.activation(eseg[:, :sc_ * 128], eseg[:, :sc_ * 128],
                                         ACTF.Copy, bias=MAGIC, scale=QS)
                    nc.scalar.activation(eseg[:, :sc_ * 128], eseg[:, :sc_ * 128],
                                         ACTF.Copy, bias=QB2, scale=QI)
                    for b in range(0, sc_, 4):
                        gs = min(4, sc_ - b)
                        ep = enc_ps.tile([128, 4 * C], F32, name="encp", tag="encp")
                        for q in range(gs):
                            nc.tensor.matmul(
                                ep[:, q * C:(q + 1) * C],
                                lhsT=eseg[:, (b + q) * 128:(b + q + 1) * 128],
                                rhs=rhs_edge[:, :], start=True, stop=True)
                        es = enc.tile([128, 4 * C], F32, name="encs2", tag="encs")
                        nc.scalar.activation(es[:, :gs * C], ep[:, :gs * C],
                                             ACTF.Copy, bias=MAGIC, scale=QS)
                        nc.vector.tensor_scalar(es[:, :gs * C], es[:, :gs * C],
                                                YMAX, YMIN, AL.min, AL.max)
                        nc.vector.tensor_tensor(es[:, :gs * C], es[:, :gs * C],
                                                scE4[:, :gs * C], op=AL.mult)
                        esb = enc.tile([128, 4 * C], BF16, name="esb", tag="esb")
                        nc.vector.tensor_tensor(esb[:, :gs * C], es[:, :gs * C],
                                                biE4[:, :gs * C], op=AL.add)
                        ch0_ = s0 + b
                        nc.sync.dma_start(
                            e_dram[:, ch0_:ch0_ + gs, :],
                            esb[:, :gs * C].rearrange("p (t c) -> p t c", c=C))

                # first AllGather
                if cfg.use_collectives:
                    nc.gpsimd.collective_compute(
                        "AllGather", AL.bypass, replica_groups=RG,
                        ins=[h64A[:, :]], outs=[h64f[0][:, :]])
                else:
                    for b_ in range(cfg.ncores):
                        nc.sync.dma_start(h64f[0][b_ * NB:(b_ + 1) * NB, :],
                                          h64A[:, :])

            # ---- layers ----
            with (
                tc.tile_pool(name="edge", bufs=3) as epool,
                tc.tile_pool(name="hsgp", bufs=3) as hsgp,
                tc.tile_pool(name="node", bufs=2) as npool,
                tc.tile_pool(name="eps", bufs=1, space="PSUM") as ps_edge,
                tc.tile_pool(name="mlp1", bufs=1, space="PSUM") as ps_z1,
                tc.tile_pool(name="mlp2", bufs=1, space="PSUM") as ps_z2,
                tc.tile_pool(name="tr", bufs=1, space="PSUM") as ps_tr,
                tc.tile_pool(name="poolps", bufs=1, space="PSUM") as ps_pool,
            ):
              for l in range(min(L, cfg.n_layers)):
                  h_in = h64A if l % 2 == 0 else h64B
                  h_out = h64B if l % 2 == 0 else h64A
                  last = l == L - 1

                  for gi, grp in enumerate(cfg.groups):
                      F = cfg.group_F[gi]
                      cmeta = cfg.chunk_meta[gi]
                      imeta = cfg.instr_meta[gi]
                      ngt = len(grp)
                      c0 = imeta[0][3]
                      c1 = imeta[-1][3] + imeta[-1][1] * 8

                      idxt = epool.tile([128, cfg.MAXCOLS], I16, name="idxt",
                                        tag="idxt", padded_shape=[128, cfg.MAXCOLS])
                      nc.sync.dma_start(idxt[:, :c1 - c0], d_gidx[:, c0:c1])
                      eps_t = [ps_edge.tile([128, C2], F32, name=f"eps{t}",
                                            tag=f"eps{t}") for t in range(ngt)]

                      # --- segment-pipelined edge phase (one gather instr each) ---
                      for (ch0, nchi, r, col0) in imeta:
                          ni = nchi * 128
                          hsg = hsgp.tile([128, GMAXCH, CE], F32, name="hsg",
                                          tag="hsg")
                          nc.gpsimd.dma_gather(
                              hsg[:, :nchi, :],
                              h64f[l][r * RSZ:(r + 1) * RSZ, :],
                              idxt[:, col0 - c0:col0 - c0 + nchi * 8],
                              ni, ni, CE, single_packet=False)
                          et = epool.tile([128, GMAXCH, C], BF16, name="et",
                                          tag="et")
                          nc.sync.dma_start(
                              et[:, :nchi, :],
                              e_dram[:, F + ch0:F + ch0 + nchi, :])
                          dlt = epool.tile([128, GMAXCH], F32, name="dlt",
                                           tag="dlt")
                          nc.sync.dma_start(
                              dlt[:, :nchi],
                              d_dloc[:, F + ch0:F + ch0 + nchi])
                          xb = epool.tile([128, GMAXCH, C], BF16, name="xb",
                                          tag="xb")
                          nc.vector.tensor_tensor(xb[:, :nchi, :],
                                                  hsg[:, :nchi, 0:C],
                                                  et[:, :nchi, :], op=AL.add)
                          exm = epool.tile([128, GMAXCH, C2], BF16, name="exm",
                                           tag="exm")
                          gx = epool.tile([128, GMAXCH, C], BF16, name="gx",
                                          tag="gx")
                          # gx = exp(t*x); den-term = max(gx,1) == exp(t*relu(x))
                          nc.scalar.activation(gx[:, :nchi, :],
                                               xb[:, :nchi, :], ACTF.Exp,
                                               scale=t_bc[:, l:l + 1])
                          nc.vector.tensor_scalar(exm[:, :nchi, C:C2],
                                                  gx[:, :nchi, :],
                                                  1.0, None, AL.max)
                          # num-term: relu(x) * gx (one fused op)
                          nc.vector.scalar_tensor_tensor(
                              exm[:, :nchi, 0:C], xb[:, :nchi, :], 0.0,
                              gx[:, :nchi, :], op0=AL.max, op1=AL.mult)
                          oh = epool.tile([128, GMAXCH, 128], BF16, name="oh",
                                          tag="oh")
                          nc.vector.tensor_tensor(
                              oh[:, :nchi, :],
                              dlt[:, :nchi].to_broadcast([128, nchi, 128]),
                              iota4[:, 0:1, :].to_broadcast([128, nchi, 128]),
                              op=AL.is_equal)
                          for q in range(nchi):
                              t_, st, sp = cmeta[ch0 + q]
                              nc.tensor.matmul(
                                  eps_t[t_][:, :], lhsT=oh[:, q, :],
                                  rhs=exm[:, q, :], start=st, stop=sp)

                      # --- node phase per tile; MLP over the group ---
                      hog = None
                      h2qT = None
                      for ti, g in enumerate(grp):
                          tq = ti % 4
                          if tq == 0:
                              gs = min(4, ngt - ti)
                              hog = npool.tile([128, 4, C], F32, name="hog",
                                               tag="hog")
                              h2qT = npool.tile([C, 512], F32, name="h2qT",
                                                tag="h2qT")
                          nc.sync.dma_start(hog[:, tq, :],
                                            h_in[g * 128:(g + 1) * 128, 0:C])
                          dinv = npool.tile([128, C], F32, name="dinv", tag="dinv")
                          nc.vector.tensor_scalar(dinv[:, :], eps_t[ti][:, C:C2],
                                                  1e-16, None, AL.max)
                          nc.vector.reciprocal(dinv[:, :], dinv[:, :])
                          h2 = npool.tile([128, C], F32, name="h2", tag="h2")
                          nc.vector.tensor_tensor(h2[:, :], eps_t[ti][:, 0:C],
                                                  dinv[:, :], op=AL.mult)
                          nc.vector.tensor_tensor(h2[:, :], h2[:, :],
                                                  hog[:, tq, :], op=AL.add)
                          emit_q(nc, h2[:, :])
                          trp = ps_tr.tile([C, 128], F32, name="trp", tag="tr")
                          nc.tensor.transpose(trp[:, :], h2[:, :],
                                              identity=ident[:, :])
                          nc.vector.tensor_copy(h2qT[:, tq * 128:(tq + 1) * 128],
                                                trp[:, :])

                          if tq == gs - 1:
                              w = gs * 128
                              g0 = g - gs + 1
                              z1p = ps_z1.tile([C2, 512], F32, name="z1p", tag="z1p")
                              nc.tensor.matmul(z1p[:, :w], lhsT=W1q[l][:, :],
                                               rhs=h2qT[:, :w], start=True, stop=True)
                              z1s = npool.tile([C2, 512], F32, name="z1s", tag="z1s")
                              nc.scalar.activation(z1s[:, :w], z1p[:, :w],
                                                   ACTF.Identity,
                                                   bias=bias1[l][:, :], scale=QS)
                              nc.scalar.activation(z1s[:, :w], z1s[:, :w], ACTF.Copy,
                                                   bias=QB2, scale=QI)
                              nc.vector.tensor_scalar(z1s[:, :w], z1s[:, :w],
                                                      QMAX, QMIN, AL.min, AL.max)
                              nc.scalar.activation(z1s[:, :w], z1s[:, :w], ACTF.Relu,
                                                   bias=bi1[l][:, :], scale=sc1[l][:, :])
                              nc.scalar.activation(z1s[:, :w], z1s[:, :w], ACTF.Copy,
                                                   bias=MAGIC, scale=QS)
                              nc.scalar.activation(z1s[:, :w], z1s[:, :w], ACTF.Copy,
                                                   bias=QB2, scale=QI)
                              nc.vector.tensor_scalar(z1s[:, :w], z1s[:, :w],
                                                      QMAX, QMIN, AL.min, AL.max)
                              z2p = ps_z2.tile([C, 512], F32, name="z2p", tag="z2p")
                              nc.tensor.matmul(z2p[:, :w], lhsT=W2q[l][:, :],
                                               rhs=z1s[:, :w], start=True, stop=True)
                              z2s = npool.tile([C, 512], F32, name="z2s", tag="z2s")
                              nc.scalar.activation(z2s[:, :w], z2p[:, :w],
                                                   ACTF.Identity,
                                                   bias=bias2[l][:, :], scale=QS)
                              nc.scalar.activation(z2s[:, :w], z2s[:, :w], ACTF.Copy,
                                                   bias=QB2, scale=QI)
                              nc.vector.tensor_scalar(z2s[:, :w], z2s[:, :w],
                                                      QMAX, QMIN, AL.min, AL.max)
                              hnext = npool.tile([128, 4, C], F32, name="hnext",
                                                 tag="hnext")
                              for q in range(gs):
                                  trq = ps_tr.tile([128, C], F32, name="trq",
                                                   tag="tr")
                                  nc.tensor.transpose(trq[:, :],
                                                      z2s[:, q * 128:(q + 1) * 128],
                                                      identity=ident[0:C, 0:C])
                                  nc.vector.tensor_tensor(hnext[:, q, :], trq[:, :],
                                                          hog[:, q, :], op=AL.add)
                                  if last:
                                      blt = npool.tile([128, 1], F32, name="blt",
                                                       tag="blt")
                                      nc.sync.dma_start(
                                          blt[:, :],
                                          d_bloc[(g0 + q) * 128:(g0 + q + 1) * 128]
                                          .rearrange("(p one) -> p one", one=1))
                                      ohp = npool.tile([128, NW, 128], F32,
                                                       name="ohp", tag="ohp")
                                      nc.vector.tensor_tensor(
                                          ohp[:, :, :],
                                          blt[:, :].to_broadcast([128, NW, 128]),
                                          iota5[:, :, :], op=AL.is_equal)
                                      for wi in range(NW):
                                          pps = ps_pool.tile([128, C], F32,
                                                             name="pps", tag="pps")
                                          nc.tensor.matmul(
                                              pps[:, :], lhsT=ohp[:, wi, :],
                                              rhs=hnext[:, q, :],
                                              start=True, stop=True)
                                          nc.vector.tensor_tensor(
                                              pacc[:, wi, :], pacc[:, wi, :],
                                              pps[:, :], op=AL.add)
                              if not last:
                                  nc.sync.dma_start(
                                      h_out[g0 * 128:(g0 + gs) * 128, 0:C]
                                      .rearrange("(t p) c -> p t c", p=128),
                                      hnext[:, :gs, :])

                  if not last:
                      if cfg.use_collectives:
                          nc.gpsimd.collective_compute(
                              "AllGather", AL.bypass, replica_groups=RG,
                              ins=[h_out[:, :]], outs=[h64f[l + 1][:, :]])
                      else:
                          for b_ in range(cfg.ncores):
                              nc.sync.dma_start(
                                  h64f[l + 1][b_ * NB:(b_ + 1) * NB, :],
                                  h_out[:, :])

              if cfg.n_layers < L:
                  nl = cfg.n_layers
                  hf = h64f[min(nl, L - 1)]
                  for b_ in range(NPAD // 128):
                      dbg_t = npool.tile([128, C], F32, name="dbg_t", tag="dbg_t")
                      nc.sync.dma_start(dbg_t[:, :],
                                        hf[b_ * 128:(b_ + 1) * 128, 0:C])
                      nc.sync.dma_start(d_hdbg[b_ * 128:(b_ + 1) * 128, :],
                                        dbg_t[:, :])
                  return nc

              # ---- pooling: write window partials, AllReduce, output head ----
              nc.sync.dma_start(
                  pool_glob[:, :].rearrange("(w p) c -> p w c", p=128),
                  pacc[:, :, :])
              if cfg.use_collectives:
                  nc.gpsimd.collective_compute(
                      "AllReduce", AL.add, replica_groups=RG,
                      ins=[pool_glob[:, :]], outs=[pool_red[:, :]])
              else:
                  nc.sync.dma_start(pool_red[:, :], pool_glob[:, :])

              n_out_tiles = (G + 127) // 128
              for i in range(n_out_tiles):
                  w = min(128, G - i * 128)
                  pt = npool.tile([128, C], F32, name="pt", tag="pt")
                  nc.sync.dma_start(pt[:w, :], pool_red[i * 128:i * 128 + w, :])
                  civ = npool.tile([128, 1], F32, name="civ", tag="civ")
                  nc.sync.dma_start(civ[:w, :],
                                    d_cntinv[i * 128:i * 128 + w].rearrange("(p one) -> p one", one=1))
                  nc.vector.tensor_scalar(pt[:w, :], pt[:w, :], civ[:w, :], None, AL.mult)
                  emit_q(nc, pt[:w, :])
                  trh = ps_tr.tile([C, 128], F32, name="trh", tag="tr")
                  nc.tensor.transpose(trh[:, :w], pt[:w, :], identity=ident[:w, :w])
                  hts = npool.tile([C, 128], F32, name="hts", tag="hts")
                  nc.vector.tensor_copy(hts[:, :w], trh[:, :w])
                  op_ = ps_z2.tile([1, 128], F32, name="op_", tag="z2p")
                  nc.tensor.matmul(op_[:, :w], lhsT=Woq[:, :], rhs=hts[:, :w],
                                   start=True, stop=True)
                  osb = npool.tile([1, 128], F32, name="osb", tag="osb")
                  nc.scalar.activation(osb[:, :w], op_[:, :w], ACTF.Identity,
                                       bias=biaso[:, :], scale=QS)
                  nc.scalar.activation(osb[:, :w], osb[:, :w], ACTF.Copy,
                                       bias=QB2, scale=QI)
                  nc.vector.tensor_scalar(osb[:, :w], osb[:, :w], QMAX, QMIN,
                                          AL.min, AL.max)
                  nc.scalar.activation(osb[:, :w], osb[:, :w], ACTF.Sigmoid)
                  nc.scalar.activation(osb[:, :w], osb[:, :w], ACTF.Copy,
                                       bias=MAGIC, scale=QS)
                  nc.scalar.activation(osb[:, :w], osb[:, :w], ACTF.Copy,
                                       bias=QB2, scale=QI)
                  nc.sync.dma_start(
                      d_out[i * 128:i * 128 + w, :].rearrange("w one -> one w"),
                      osb[:, :w])

    return nc


# ----------------------------------------------------------------------------
# Entry point.
# ----------------------------------------------------------------------------

def run(inputs, cfg, **run_kwargs):
    global LAST_RESULTS
    in_maps = preprocess(inputs, cfg)
    nc = build(cfg)
    if not nc.is_finalized():
        nc.finalize()
    res = run_bass_kernel_spmd(nc, in_maps, core_ids=list(range(cfg.ncores)),
                               **run_kwargs)
    LAST_RESULTS = res
    return res.results[0]["out"].reshape(cfg.G, 1).astype(np.float32)


def kernel(**inputs) -> np.ndarray:
    cfg = Cfg(N=100000, E=3200000, G=512, XD=8, ED=4, C=32, L=4)
    return run(inputs, cfg)


# revision 31
# speedup vs baseline: 1.5857x; 1.3407x over previous
"""Trainium2 Bass kernel: nn_BV_Model (GENConv GNN, softmax aggregation, 4 layers).

Strategy (8 NeuronCores, SPMD), v2:
  - Nodes partitioned into 8 contiguous blocks (12544/core, padded); edges
    sorted by destination and bucketed per destination node-tile (128 nodes).
  - h is replicated per-core in DRAM as fp32 [NPAD, 64] (channels padded
    32->64 so one node row is 256B) and re-AllGathered per layer.
  - The per-edge h[src] gather uses the batched SWDGE dma_gather
    (InstDMAGatherAnt): 1024 edges per instruction (vs. one 128-edge chunk
    per indirect_dma_start), with edges grouped per (8-tile supertile,
    src-range) so int16 indices stay in range (4 ranges of NPAD/4 rows).
  - Edge math: x = h_src + e; with t>=0, exp(t*relu(x)) == max(exp(t*x), 1),
    so num-term relu(x)*ex is one fused scalar_tensor_tensor op and den-term
    is one tensor_scalar max. eps=1e-7 is dropped (error ~1e-7 << tol).
  - Segment softmax reduced edges->nodes with a one-hot(dst) matmul in bf16
    accumulated in PSUM; e / exmex / one-hot are bf16 (tol 2e-2).
  - Node MLP on the tensor engine in transposed layout (fp32, exact quant).
  - Global mean pool per-core with one-hot(graph) matmuls + AllReduce.

Fake-quantization q(x) = clip(rne(x*1024), -32768, 32767)/1024 via the
round-to-nearest-even "magic number" trick (+1.5*2^23).
"""

import os
os.environ.setdefault("MYCRO_LOCAL_CACHE", "1")

import math
import numpy as np

import concourse.bacc as bacc
import concourse.tile as tile
import concourse.bass as bass
from concourse import mybir
from concourse import library_config
from concourse.bass_utils import run_bass_kernel_spmd

F32 = mybir.dt.float32
BF16 = mybir.dt.bfloat16
I16 = mybir.dt.int16
I32 = mybir.dt.int32
ACTF = mybir.ActivationFunctionType
AL = mybir.AluOpType

MAGIC = 12582912.0           # 1.5*2^23 : fp32 RNE rounding magic
QS = 1024.0                  # 2^10
QI = 1.0 / 1024.0
QB2 = -12288.0               # -MAGIC * 2^-10
QMAX = 32767.0 / 1024.0
QMIN = -32.0
# clip bounds in the y = 1024*q + MAGIC domain (pre-descale)
YMAX = QMAX * QS + MAGIC
YMIN = QMIN * QS + MAGIC
BN_EPS = 1e-5
NCORES = 8
GT = 4                       # tiles per gather/edge supertile (= MLP group)
GMAXCH = 8                   # max chunks (of 128 idx) per dma_gather instr

LAST_RESULTS = None          # BassKernelResults of the most recent run


class Cfg:
    def __init__(self, N, E, G, XD=8, ED=4, C=32, L=4, ncores=NCORES,
                 use_collectives=True, use_shared=True, gather_k=None,
                 no_indirect=False, gather_plain=True, n_layers=None):
        self.N, self.E, self.G = N, E, G
        self.XD, self.ED, self.C, self.L = XD, ED, C, L
        self.ncores = ncores
        self.use_collectives = use_collectives and ncores > 1
        self.use_shared = use_shared
        self.n_layers = L if n_layers is None else n_layers
        self.TPC = (N + ncores * 128 - 1) // (ncores * 128)    # node tiles per core
        self.NB = self.TPC * 128                               # nodes per core (padded)
        self.NPAD = self.NB * ncores
        self.NR = 4
        assert self.NPAD % self.NR == 0
        self.RSZ = self.NPAD // self.NR                        # src range rows
        assert self.RSZ <= 32767
        self.PG = ((G + 128) + 127) // 128 * 128               # pooled scatter rows
        # filled by preprocess:
        self.groups = None          # list[list[g]]
        self.chunk_meta = None      # per group: list[(t_in_group, start, stop)]
        self.instr_meta = None      # per group: list[(ch0_local, nch, r, col0)]
        self.group_F = None         # per group: global chunk offset
        self.NCHG = None            # max chunks in a group
        self.SLOTS = None
        self.SCOLS = None


# ----------------------------------------------------------------------------
# Host-side preprocessing: sort/bucket edges, build per-core input arrays.
# ----------------------------------------------------------------------------

def preprocess(inputs, cfg):
    x = np.ascontiguousarray(np.asarray(inputs["x"], np.float32))
    ea = np.ascontiguousarray(np.asarray(inputs["edge_attr"], np.float32))
    ei = np.asarray(inputs["edge_index"]).astype(np.int64)
    batch = np.asarray(inputs["batch"]).astype(np.int64)
    N, E, G = cfg.N, cfg.E, cfg.G
    XD, ED, C, L = cfg.XD, cfg.ED, cfg.C, cfg.L
    TPC, NB, NR, RSZ = cfg.TPC, cfg.NB, cfg.NR, cfg.RSZ
    NC_ = cfg.ncores

    src, dst = ei[0], ei[1]
    order = np.argsort(dst, kind="stable")
    src_s = src[order]
    dst_s = dst[order]
    ea_s = ea[order]

    ntiles = NC_ * TPC
    bnd = np.searchsorted(dst_s, np.arange(ntiles + 1) * 128)

    # per (core, tile): reorder edges by src range; count per range
    cnt = np.zeros((NC_, TPC, NR), np.int64)
    seg = {}                       # (c, g, r) -> (src_rel, dloc, ea rows)
    for c in range(NC_):
        for g in range(TPC):
            t = c * TPC + g
            b0, b1 = int(bnd[t]), int(bnd[t + 1])
            if b1 <= b0:
                continue
            s = src_s[b0:b1]
            d = dst_s[b0:b1]
            e_ = ea_s[b0:b1]
            r = s // RSZ
            o = np.argsort(r, kind="stable")
            s, d, e_, r = s[o], d[o], e_[o], r[o]
            rb = np.searchsorted(r, np.arange(NR + 1))
            for rr in range(NR):
                m = int(rb[rr + 1] - rb[rr])
                cnt[c, g, rr] = m
                if m:
                    sl = slice(int(rb[rr]), int(rb[rr + 1]))
                    seg[(c, g, rr)] = (
                        (s[sl] - rr * RSZ).astype(np.int16),
                        (d[sl] - t * 128).astype(np.float32),
                        e_[sl],
                    )

    K = (cnt.max(axis=0) + 127) // 128                         # [TPC, NR]

    # group structure + chunk/instr tables (shared across cores)
    groups = [list(range(g0, min(g0 + GT, TPC))) for g0 in range(0, TPC, GT)]
    chunk_meta, instr_meta, group_F = [], [], []
    ch_start = {}                  # (g, r) -> global chunk index
    ch = 0
    for grp in groups:
        group_F.append(ch)
        cm = []
        im = []
        # first/last chunk index (local) per tile for start/stop flags
        tile_chunks = {ti: [] for ti in range(len(grp))}
        local = 0
        runs = []
        for r in range(NR):
            run0 = local
            for ti, g in enumerate(grp):
                for _ in range(int(K[g, r])):
                    cm.append([ti, False, False])
                    tile_chunks[ti].append(local)
                    local += 1
            runs.append((run0, local - run0, r))
        for ti, lst in tile_chunks.items():
            if lst:
                cm[lst[0]][1] = True
                cm[lst[-1]][2] = True
        for (run0, n, r) in runs:
            o = 0
            while o < n:
                nch = min(GMAXCH, n - o)
                im.append([run0 + o, nch, r, 0])
                o += nch
        chunk_meta.append([tuple(e) for e in cm])
        instr_meta.append(im)
        for r in range(NR):
            for g in grp:
                ch_start[(g, r)] = ch
                ch += int(K[g, r])
    NCH_TOT = ch
    SLOTS = NCH_TOT * 128
    # idx column offsets (global, shared)
    col = 0
    maxcols = 0
    for gi in range(len(groups)):
        col0g = col
        for e in instr_meta[gi]:
            e[3] = col
            col += e[1] * 8  # nch*128/16 = nch*8 columns
        maxcols = max(maxcols, col - col0g)
        instr_meta[gi] = [tuple(e) for e in instr_meta[gi]]
    SCOLS = col

    cfg.groups, cfg.chunk_meta, cfg.instr_meta = groups, chunk_meta, instr_meta
    cfg.group_F = group_F
    cfg.NCHG = max(len(cm) for cm in chunk_meta)
    cfg.MAXCOLS = maxcols
    cfg.SLOTS, cfg.SCOLS = SLOTS, SCOLS
    cfg.NCH_TOT = NCH_TOT

    # per-core slot arrays
    idxrel_a = np.zeros((NC_, SLOTS), np.int16)
    dloc_a = np.full((NC_, SLOTS), -1.0, np.float32)
    eaT_a = np.zeros((NC_, ED + 1, SLOTS), np.float32)
    eaT_a[:, ED, :] = 1.0
    for c in range(NC_):
        for g in range(TPC):
            for r in range(NR):
                m = int(cnt[c, g, r])
                if m == 0:
                    continue
                s0 = ch_start[(g, r)] * 128
                sr, dl, er = seg[(c, g, r)]
                idxrel_a[c, s0:s0 + m] = sr
                dloc_a[c, s0:s0 + m] = dl
                eaT_a[c, :ED, s0:s0 + m] = er.T
    # dloc in partition-major [128, NCH_TOT] layout (slot = ch*128 + p)
    dloc_pm = np.ascontiguousarray(
        dloc_a.reshape(NC_, NCH_TOT, 128).transpose(0, 2, 1))

    # wrapped int16 index arrays per instruction
    idx16_a = np.zeros((NC_, 128, SCOLS), np.int16)
    for gi, grp in enumerate(groups):
        F = group_F[gi]
        for (ch0, nch, r, col0) in instr_meta[gi]:
            ni = nch * 128
            s0 = (F + ch0) * 128
            for c in range(NC_):
                flat = idxrel_a[c, s0:s0 + ni]
                blk = flat.reshape(ni // 16, 16).T      # [16, ni/16]
                idx16_a[c, :, col0:col0 + ni // 16] = np.tile(blk, (8, 1))

    xT_a = np.zeros((NC_, XD + 1, NB), np.float32)
    xT_a[:, XD, :] = 1.0
    bloc_a = np.full((NC_, NB), -1.0, np.float32)
    first_g = np.zeros(NC_, np.int64)
    for c in range(NC_):
        lo, hi = c * NB, min((c + 1) * NB, N)
        xT_a[c, :XD, : hi - lo] = x[lo:hi].T
        first_g[c] = batch[lo]
        assert batch[hi - 1] - batch[lo] < 128, "graph window exceeds 128"
        bloc_a[c, : hi - lo] = batch[lo:hi].astype(np.float32)

    prow_a = (first_g[:, None] + np.arange(128)[None, :]).astype(np.int32)
    assert prow_a.max() < cfg.PG
    cnt_g = np.bincount(batch, minlength=G).astype(np.float32)
    cnt_inv = np.zeros(cfg.PG, np.float32)
    cnt_inv[:G] = np.float32(1.0) / np.maximum(cnt_g, np.float32(1.0))

    def f32(a):
        return np.ascontiguousarray(np.asarray(a, np.float32))

    shared = dict(
        W_node=f32(inputs["W_node"]), b_node=f32(inputs["b_node"]).reshape(1, C),
        W_edge=f32(inputs["W_edge"]), b_edge=f32(inputs["b_edge"]).reshape(1, C),
        bnn_g=f32(inputs["bnn_g"]).reshape(1, C), bnn_b=f32(inputs["bnn_b"]).reshape(1, C),
        bnn_m=f32(inputs["bnn_m"]).reshape(1, C), bnn_v=f32(inputs["bnn_v"]).reshape(1, C),
        bne_g=f32(inputs["bne_g"]).reshape(1, C), bne_b=f32(inputs["bne_b"]).reshape(1, C),
        bne_m=f32(inputs["bne_m"]).reshape(1, C), bne_v=f32(inputs["bne_v"]).reshape(1, C),
        t=f32(inputs["t"]).reshape(1, L),
        W1=f32(inputs["W1"]), b1=f32(inputs["b1"]),
        bn1_g=f32(inputs["bn1_g"]), bn1_b=f32(inputs["bn1_b"]),
        bn1_m=f32(inputs["bn1_m"]), bn1_v=f32(inputs["bn1_v"]),
        W2=f32(inputs["W2"]), b2=f32(inputs["b2"]),
        W_out=f32(inputs["W_out"]), b_out=f32(inputs["b_out"]).reshape(1, 1),
        cnt_inv=cnt_inv,
    )
    in_maps = []
    for c in range(NC_):
        im = dict(shared)
        im.update(
            xT=xT_a[c], eaT=eaT_a[c].reshape(-1), gidx16=idx16_a[c],
            dloc=dloc_pm[c], bloc=bloc_a[c], prow=prow_a[c],
        )
        in_maps.append(im)
    return in_maps


# ----------------------------------------------------------------------------
# Device program.
# ----------------------------------------------------------------------------

def emit_q(nc, ap, pre_bias_ap=None, clip=True):
    """In-place fake quantization of `ap` (fp32): q(x) (+fused bias if given)."""
    if pre_bias_ap is None:
        nc.scalar.activation(ap, ap, ACTF.Copy, bias=MAGIC, scale=QS)
    else:
        nc.scalar.activation(ap, ap, ACTF.Identity, bias=pre_bias_ap, scale=QS)
    nc.scalar.activation(ap, ap, ACTF.Copy, bias=QB2, scale=QI)
    if clip:
        nc.vector.tensor_scalar(ap, ap, QMAX, QMIN, AL.min, AL.max)


def build(cfg):
    C, L, TPC, NB = cfg.C, cfg.L, cfg.TPC, cfg.NB
    XD, ED, G, PG = cfg.XD, cfg.ED, cfg.G, cfg.PG
    NPAD, NR, RSZ = cfg.NPAD, cfg.NR, cfg.RSZ
    SLOTS, SCOLS, NCHG = cfg.SLOTS, cfg.SCOLS, cfg.NCHG
    C2 = 2 * C
    CE = 64                                          # padded h row (fp32)
    RG = [list(range(cfg.ncores))]
    SHARED = "Shared" if (cfg.use_shared and cfg.use_collectives) else "Local"

    nc = bacc.Bacc("TRN2", target_bir_lowering=False, debug=False,
                   enable_asserts=False, num_devices=cfg.ncores,
                   num_swdge_queues=4)

    # ---- kernel I/O ----
    NCH_TOT = cfg.NCH_TOT
    d_xT = nc.dram_tensor("xT", [XD + 1, NB], F32, kind="ExternalInput")
    d_eaT = nc.dram_tensor("eaT", [(ED + 1) * SLOTS], F32, kind="ExternalInput")
    d_gidx = nc.dram_tensor("gidx16", [128, SCOLS], I16, kind="ExternalInput")
    d_dloc = nc.dram_tensor("dloc", [128, NCH_TOT], F32, kind="ExternalInput")
    d_bloc = nc.dram_tensor("bloc", [NB], F32, kind="ExternalInput")
    d_prow = nc.dram_tensor("prow", [128], I32, kind="ExternalInput")
    d_cntinv = nc.dram_tensor("cnt_inv", [PG], F32, kind="ExternalInput")
    d_Wn = nc.dram_tensor("W_node", [XD, C], F32, kind="ExternalInput")
    d_bn_ = nc.dram_tensor("b_node", [1, C], F32, kind="ExternalInput")
    d_We = nc.dram_tensor("W_edge", [ED, C], F32, kind="ExternalInput")
    d_be = nc.dram_tensor("b_edge", [1, C], F32, kind="ExternalInput")
    d_bnr = {k: nc.dram_tensor(k, [1, C], F32, kind="ExternalInput")
             for k in ["bnn_g", "bnn_b", "bnn_m", "bnn_v",
                       "bne_g", "bne_b", "bne_m", "bne_v"]}
    d_t = nc.dram_tensor("t", [1, L], F32, kind="ExternalInput")
    d_W1 = nc.dram_tensor("W1", [L, C, C2], F32, kind="ExternalInput")
    d_b1 = nc.dram_tensor("b1", [L, C2], F32, kind="ExternalInput")
    d_bn1 = {k: nc.dram_tensor(k, [L, C2], F32, kind="ExternalInput")
             for k in ["bn1_g", "bn1_b", "bn1_m", "bn1_v"]}
    d_W2 = nc.dram_tensor("W2", [L, C2, C], F32, kind="ExternalInput")
    d_b2 = nc.dram_tensor("b2", [L, C], F32, kind="ExternalInput")
    d_Wo = nc.dram_tensor("W_out", [C, 1], F32, kind="ExternalInput")
    d_bo = nc.dram_tensor("b_out", [1, 1], F32, kind="ExternalInput")
    d_out = nc.dram_tensor("out", [G, 1], F32, kind="ExternalOutput")
    d_hdbg = nc.dram_tensor("h_dbg", [NPAD, C], F32, kind="ExternalOutput")

    # ---- inline constants ----
    eye = np.eye(128, dtype=np.float32)
    iota4_np = np.tile(np.arange(128, dtype=np.float32), (128, 4, 1))
    ones_np = np.ones((1, 128), np.float32)
    c_eye = nc.inline_tensor(eye, "c_eye")
    c_iota4 = nc.inline_tensor(iota4_np, "c_iota4")
    NW = PG // 128                                   # pooling windows
    iota5_np = (np.tile(np.arange(128, dtype=np.float32), (128, NW, 1))
                + (np.arange(NW, dtype=np.float32) * 128)[None, :, None])
    c_iota5 = nc.inline_tensor(iota5_np, "c_iota5")
    c_ones = nc.inline_tensor(ones_np, "c_ones")

    with tile.TileContext(nc) as tc:
        with (
            tc.tile_pool(name="dram", bufs=1, space="DRAM") as dpool,
            tc.tile_pool(name="const", bufs=1) as cp,
        ):
            # ---- internal DRAM ----
            h64A = dpool.tile([NB, CE], F32, name="h64A")
            h64B = dpool.tile([NB, CE], F32, name="h64B")
            h64f = [dpool.tile([NPAD, CE], F32, addr_space=SHARED,
                               name=f"h64f_{l}") for l in range(L)]
            e_dram = dpool.tile([128, NCH_TOT, C], BF16, name="e_dram")
            xq_dram = dpool.tile([(XD + 1) * NB], F32, name="xq_dram")
            pool_glob = dpool.tile([PG, C], F32, name="pool_glob")
            pool_red = dpool.tile([PG, C], F32, addr_space=SHARED, name="pool_red")

            # ---- constants to SBUF ----
            nc.gpsimd.load_library(library_config.mlp)
            ident = cp.tile([128, 128], F32, name="ident")
            nc.sync.dma_start(ident[:, :], c_eye[:, :])
            iota4 = cp.tile([128, 4, 128], F32, name="iota4")
            nc.sync.dma_start(iota4[:, :, :], c_iota4[:, :, :])
            iota5 = cp.tile([128, NW, 128], F32, name="iota5")
            nc.sync.dma_start(iota5[:, :, :], c_iota5[:, :, :])
            pacc = cp.tile([128, NW, C], F32, name="pacc")
            nc.vector.memset(pacc[:, :, :], 0.0)
            onesr = cp.tile([1, 128], F32, name="onesr")
            nc.sync.dma_start(onesr[:, :], c_ones[:, :])

            # zero-fill h64 local buffers once (pad columns stay 0 forever)
            zt = cp.tile([128, 8, CE], F32, name="zt")
            nc.vector.memset(zt[:, :, :], 0.0)
            for hb in (h64A, h64B):
                for b in range(0, TPC, 8):
                    gs_ = min(8, TPC - b)
                    nc.sync.dma_start(
                        hb[b * 128:(b + gs_) * 128, :]
                        .rearrange("(t p) c -> p t c", p=128),
                        zt[:, :gs_, :])

            # ---- parameter prep ----
            rhs_node = cp.tile([XD + 1, C], F32, name="rhs_node")
            nc.sync.dma_start(rhs_node[:XD, :], d_Wn[:, :])
            nc.sync.dma_start(rhs_node[XD:XD + 1, :], d_bn_[:, :])
            emit_q(nc, rhs_node[:, :])
            rhs_edge = cp.tile([ED + 1, C], F32, name="rhs_edge")
            nc.sync.dma_start(rhs_edge[:ED, :], d_We[:, :])
            nc.sync.dma_start(rhs_edge[ED:ED + 1, :], d_be[:, :])
            emit_q(nc, rhs_edge[:, :])

            def bn_rows2(pref):
                g_ = cp.tile([1, C], F32, name=pref + "_g")
                b_ = cp.tile([1, C], F32, name=pref + "_b")
                m_ = cp.tile([1, C], F32, name=pref + "_m")
                sc = cp.tile([1, C], F32, name=pref + "_sc")
                bi = cp.tile([1, C], F32, name=pref + "_bi")
                tmp = cp.tile([1, C], F32, name=pref + "_tmp")
                nc.sync.dma_start(g_[:, :], d_bnr[pref + "_g"][:, :])
                nc.sync.dma_start(b_[:, :], d_bnr[pref + "_b"][:, :])
                nc.sync.dma_start(m_[:, :], d_bnr[pref + "_m"][:, :])
                nc.sync.dma_start(sc[:, :], d_bnr[pref + "_v"][:, :])
                nc.vector.tensor_scalar(sc[:, :], sc[:, :], BN_EPS, None, AL.add)
                nc.scalar.activation(sc[:, :], sc[:, :], ACTF.Sqrt)
                nc.vector.reciprocal(sc[:, :], sc[:, :])
                nc.vector.tensor_tensor(sc[:, :], sc[:, :], g_[:, :], op=AL.mult)
                nc.vector.tensor_tensor(bi[:, :], m_[:, :], sc[:, :], op=AL.mult)
                nc.vector.tensor_tensor(bi[:, :], b_[:, :], bi[:, :], op=AL.subtract)
                # fold q second step into BN: y = 1024*q+MAGIC
                # bn(q) = q*sc + bi = y*(sc/1024) + (bi - 12288*sc)
                nc.vector.tensor_scalar(tmp[:, :], sc[:, :], -12288.0, None, AL.mult)
                nc.vector.tensor_tensor(bi[:, :], bi[:, :], tmp[:, :], op=AL.add)
                nc.vector.tensor_scalar(sc[:, :], sc[:, :], QI, None, AL.mult)
                return sc, bi

            scN, biN = bn_rows2("bnn")
            scE, biE = bn_rows2("bne")

            def replicate4(row, nm, pool):
                ps = pool.tile([128, C], F32, name="rep_ps", tag="encp")
                nc.tensor.matmul(ps[:, :], lhsT=onesr[:, :], rhs=row[:, :],
                                 start=True, stop=True)
                out4 = cp.tile([128, 4 * C], F32, name=nm)
                for q in range(4):
                    nc.vector.tensor_copy(out4[:, q * C:(q + 1) * C], ps[:, :])
                return out4

            W1q, bias1, sc1, bi1, W2q, bias2 = [], [], [], [], [], []
            for l in range(L):
                w1 = cp.tile([C, C2], F32, name=f"W1q_{l}")
                nc.sync.dma_start(w1[:, :], d_W1[l, :, :])
                emit_q(nc, w1[:, :])
                W1q.append(w1)
                b1t = cp.tile([C2, 1], F32, name=f"bias1_{l}")
                nc.sync.dma_start(b1t[:, :], d_b1[l:l + 1, :].rearrange("a b -> b a"))
                emit_q(nc, b1t[:, :])
                nc.vector.tensor_scalar(b1t[:, :], b1t[:, :], QS, MAGIC, AL.mult, AL.add)
                bias1.append(b1t)

                g1 = cp.tile([C2, 1], F32, name=f"g1_{l}")
                bb1 = cp.tile([C2, 1], F32, name=f"bb1_{l}")
                m1 = cp.tile([C2, 1], F32, name=f"m1_{l}")
                s1 = cp.tile([C2, 1], F32, name=f"sc1_{l}")
                i1 = cp.tile([C2, 1], F32, name=f"bi1_{l}")
                nc.sync.dma_start(g1[:, :], d_bn1["bn1_g"][l:l + 1, :].rearrange("a b -> b a"))
                nc.sync.dma_start(bb1[:, :], d_bn1["bn1_b"][l:l + 1, :].rearrange("a b -> b a"))
                nc.sync.dma_start(m1[:, :], d_bn1["bn1_m"][l:l + 1, :].rearrange("a b -> b a"))
                nc.sync.dma_start(s1[:, :], d_bn1["bn1_v"][l:l + 1, :].rearrange("a b -> b a"))
                nc.vector.tensor_scalar(s1[:, :], s1[:, :], BN_EPS, None, AL.add)
                nc.scalar.activation(s1[:, :], s1[:, :], ACTF.Sqrt)
                nc.vector.reciprocal(s1[:, :], s1[:, :])
                nc.vector.tensor_tensor(s1[:, :], s1[:, :], g1[:, :], op=AL.mult)
                nc.vector.tensor_tensor(i1[:, :], m1[:, :], s1[:, :], op=AL.mult)
                nc.vector.tensor_tensor(i1[:, :], bb1[:, :], i1[:, :], op=AL.subtract)
                sc1.append(s1)
                bi1.append(i1)

                w2 = cp.tile([C2, C], F32, name=f"W2q_{l}")
                nc.sync.dma_start(w2[:, :], d_W2[l, :, :])
                emit_q(nc, w2[:, :])
                W2q.append(w2)
                b2t = cp.tile([C, 1], F32, name=f"bias2_{l}")
                nc.sync.dma_start(b2t[:, :], d_b2[l:l + 1, :].rearrange("a b -> b a"))
                emit_q(nc, b2t[:, :])
                nc.vector.tensor_scalar(b2t[:, :], b2t[:, :], QS, MAGIC, AL.mult, AL.add)
                bias2.append(b2t)

            Woq = cp.tile([C, 1], F32, name="Woq")
            nc.sync.dma_start(Woq[:, :], d_Wo[:, :])
            emit_q(nc, Woq[:, :])
            biaso = cp.tile([1, 1], F32, name="biaso")
            nc.sync.dma_start(biaso[:, :], d_bo[:, :])
            emit_q(nc, biaso[:, :])
            nc.vector.tensor_scalar(biaso[:, :], biaso[:, :], QS, MAGIC, AL.mult, AL.add)

            # ---- encoders ----
            def q_pass(src_flat, dst_flat, total, pool):
                per = total // 128
                assert total % 128 == 0
                W = min(per, 4096)
                n = (per + W - 1) // W
                sv = src_flat.rearrange("(p q) -> p q", p=128)
                dv = dst_flat.rearrange("(p q) -> p q", p=128)
                for i in range(n):
                    w = min(W, per - i * W)
                    tl = pool.tile([128, W], F32, tag="qpass", name="qpass")
                    nc.sync.dma_start(tl[:, :w], sv[:, i * W:i * W + w])
                    nc.scalar.activation(tl[:, :w], tl[:, :w], ACTF.Copy,
                                         bias=MAGIC, scale=QS)
                    nc.scalar.activation(tl[:, :w], tl[:, :w], ACTF.Copy,
                                         bias=QB2, scale=QI)
                    nc.sync.dma_start(dv[:, i * W:i * W + w], tl[:, :w])

            with (
                tc.tile_pool(name="enc", bufs=2) as enc,
                tc.tile_pool(name="encx", bufs=1) as encx,
                tc.tile_pool(name="encps", bufs=2, space="PSUM") as enc_ps,
            ):
                scN4 = replicate4(scN, "scN4", enc_ps)
                biN4 = replicate4(biN, "biN4", enc_ps)
                scE4 = replicate4(scE, "scE4", enc_ps)
                biE4 = replicate4(biE, "biE4", enc_ps)

                t_sb = cp.tile([1, L], F32, name="t_sb")
                nc.sync.dma_start(t_sb[:, :], d_t[:, :])
                t_ps = enc_ps.tile([128, L], F32, name="t_ps", tag="encp")
                nc.tensor.matmul(t_ps[:, :], lhsT=onesr[:, :], rhs=t_sb[:, :],
                                 start=True, stop=True)
                t_bc = cp.tile([128, L], F32, name="t_bc")
                nc.vector.tensor_copy(t_bc[:, :], t_ps[:, :])

                # node encoder (writes h64A cols 0:32, y-domain clip + foldedBN)
                q_pass(d_xT[:, :].rearrange("a b -> (a b)"), xq_dram[:],
                       (XD + 1) * NB, enc)
                xseg = encx.tile([XD + 1, NB], F32, name="xseg")
                nc.sync.dma_start(
                    xseg[:, :], xq_dram[:].rearrange("(r e) -> r e", r=XD + 1))
                for b in range(0, TPC, 4):
                    gs = min(4, TPC - b)
                    ep = enc_ps.tile([128, 4 * C], F32, name="encp", tag="encp")
                    for q in range(gs):
                        nc.tensor.matmul(
                            ep[:, q * C:(q + 1) * C],
                            lhsT=xseg[:, (b + q) * 128:(b + q + 1) * 128],
                            rhs=rhs_node[:, :], start=True, stop=True)
                    es = enc.tile([128, 4 * C], F32, name="encs", tag="encs")
                    # y = 1024*z + MAGIC (RNE snap)
                    nc.scalar.activation(es[:, :gs * C], ep[:, :gs * C], ACTF.Copy,
                                         bias=MAGIC, scale=QS)
                    nc.vector.tensor_scalar(es[:, :gs * C], es[:, :gs * C],
                                            YMAX, YMIN, AL.min, AL.max)
                    nc.vector.tensor_tensor(es[:, :gs * C], es[:, :gs * C],
                                            scN4[:, :gs * C], op=AL.mult)
                    nc.vector.tensor_tensor(es[:, :gs * C], es[:, :gs * C],
                                            biN4[:, :gs * C], op=AL.add)
                    nc.sync.dma_start(
                        h64A[b * 128:(b + gs) * 128, 0:C]
                        .rearrange("(t p) c -> p t c", p=128),
                        es[:, :gs * C].rearrange("p (t c) -> p t c", c=C))

                # edge encoder: quantize in SBUF, matmul, foldedBN, write bf16
                eav = d_eaT[:].rearrange("(r e) -> r e", r=ED + 1)
                n_ch = SLOTS // 128
                SEGC = 32
                for s0 in range(0, n_ch, SEGC):
                    sc_ = min(SEGC, n_ch - s0)
                    eseg = enc.tile([ED + 1, SEGC * 128], F32, name="eseg",
                                    tag="eseg")
                    nc.sync.dma_start(eseg[:, :sc_ * 128],
                                      eav[:, s0 * 128:(s0 + sc_) * 128])
                    nc.scalar.activation(eseg[:, :sc_ * 128], eseg[:, :sc_ * 128],
                                         ACTF.Copy, bias=MAGIC, scale=QS)
                    nc.scalar.activation(eseg[:, :sc_ * 128], eseg[:, :sc_ * 128],
                                         ACTF.Copy, bias=QB2, scale=QI)
                    for b in range(0, sc_, 4):
                        gs = min(4, sc_ - b)
                        ep = enc_ps.tile([128, 4 * C], F32, name="encp", tag="encp")
                        for q in range(gs):
                            nc.tensor.matmul(
                                ep[:, q * C:(q + 1) * C],
                                lhsT=eseg[:, (b + q) * 128:(b + q + 1) * 128],
                                rhs=rhs_edge[:, :], start=True, stop=True)
                        es = enc.tile([128, 4 * C], F32, name="encs2", tag="encs")
                        nc.scalar.activation(es[:, :gs * C], ep[:, :gs * C],
                                             ACTF.Copy, bias=MAGIC, scale=QS)
                        nc.vector.tensor_scalar(es[:, :gs * C], es[:, :gs * C],
                                                YMAX, YMIN, AL.min, AL.max)
                        nc.vector.tensor_tensor(es[:, :gs * C], es[:, :gs * C],
                                                scE4[:, :gs * C], op=AL.mult)
                        esb = enc.tile([128, 4 * C], BF16, name="esb", tag="esb")
                        nc.vector.tensor_tensor(esb[:, :gs * C], es[:, :gs * C],
                                                biE4[:, :gs * C], op=AL.add)
                        ch0_ = s0 + b
                        nc.sync.dma_start(
                            e_dram[:, ch0_:ch0_ + gs, :],
                            esb[:, :gs * C].rearrange("p (t c) -> p t c", c=C))

                # first AllGather
                if cfg.use_collectives:
                    nc.gpsimd.collective_compute(
                        "AllGather", AL.bypass, replica_groups=RG,
                        ins=[h64A[:, :]], outs=[h64f[0][:, :]])
                else:
                    for b_ in range(cfg.ncores):
                        nc.sync.dma_start(h64f[0][b_ * NB:(b_ + 1) * NB, :],
                                          h64A[:, :])

            # ---- layers ----
            with (
                tc.tile_pool(name="edge", bufs=3) as epool,
                tc.tile_pool(name="hsgp", bufs=3) as hsgp,
                tc.tile_pool(name="node", bufs=2) as npool,
                tc.tile_pool(name="eps", bufs=1, space="PSUM") as ps_edge,
                tc.tile_pool(name="mlp1", bufs=1, space="PSUM") as ps_z1,
                tc.tile_pool(name="mlp2", bufs=1, space="PSUM") as ps_z2,
                tc.tile_pool(name="tr", bufs=1, space="PSUM") as ps_tr,
                tc.tile_pool(name="poolps", bufs=1, space="PSUM") as ps_pool,
            ):
              for l in range(min(L, cfg.n_layers)):
                  h_in = h64A if l % 2 == 0 else h64B
                  h_out = h64B if l % 2 == 0 else h64A
                  last = l == L - 1

                  for gi, grp in enumerate(cfg.groups):
                      F = cfg.group_F[gi]
                      cmeta = cfg.chunk_meta[gi]
                      imeta = cfg.instr_meta[gi]
                      ngt = len(grp)
                      c0 = imeta[0][3]
                      c1 = imeta[-1][3] + imeta[-1][1] * 8

                      idxt = epool.tile([128, cfg.MAXCOLS], I16, name="idxt",
                                        tag="idxt", padded_shape=[128, cfg.MAXCOLS])
                      nc.sync.dma_start(idxt[:, :c1 - c0], d_gidx[:, c0:c1])
                      eps_t = [ps_edge.tile([128, C2], F32, name=f"eps{t}",
                                            tag=f"eps{t}") for t in range(ngt)]

                      # --- segment-pipelined edge phase (one gather instr each) ---
                      for si, (ch0, nchi, r, col0) in enumerate(imeta):
                          ni = nchi * 128
                          hsg = hsgp.tile([128, GMAXCH, CE], F32, name="hsg",
                                          tag="hsg")
                          nc.gpsimd.dma_gather(
                              hsg[:, :nchi, :],
                              h64f[l][r * RSZ:(r + 1) * RSZ, :],
                              idxt[:, col0 - c0:col0 - c0 + nchi * 8],
                              ni, ni, CE, single_packet=False,
                              queue_num=si % 4)
                          et = epool.tile([128, GMAXCH, C], BF16, name="et",
                                          tag="et")
                          nc.sync.dma_start(
                              et[:, :nchi, :],
                              e_dram[:, F + ch0:F + ch0 + nchi, :])
                          dlt = epool.tile([128, GMAXCH], F32, name="dlt",
                                           tag="dlt")
                          nc.sync.dma_start(
                              dlt[:, :nchi],
                              d_dloc[:, F + ch0:F + ch0 + nchi])
                          xb = epool.tile([128, GMAXCH, C], BF16, name="xb",
                                          tag="xb")
                          nc.vector.tensor_tensor(xb[:, :nchi, :],
                                                  hsg[:, :nchi, 0:C],
                                                  et[:, :nchi, :], op=AL.add)
                          exm = epool.tile([128, GMAXCH, C2], BF16, name="exm",
                                           tag="exm")
                          gx = epool.tile([128, GMAXCH, C], BF16, name="gx",
                                          tag="gx")
                          # gx = exp(t*x); den-term = max(gx,1) == exp(t*relu(x))
                          nc.scalar.activation(gx[:, :nchi, :],
                                               xb[:, :nchi, :], ACTF.Exp,
                                               scale=t_bc[:, l:l + 1])
                          # (scalar_tensor_tensor: tensor_scalar lowers slow)
                          nc.vector.scalar_tensor_tensor(
                              exm[:, :nchi, C:C2], gx[:, :nchi, :], 1.0,
                              gx[:, :nchi, :], op0=AL.max, op1=AL.bypass)
                          # num-term: relu(x) * gx (one fused op)
                          nc.vector.scalar_tensor_tensor(
                              exm[:, :nchi, 0:C], xb[:, :nchi, :], 0.0,
                              gx[:, :nchi, :], op0=AL.max, op1=AL.mult)
                          oh = epool.tile([128, GMAXCH, 128], BF16, name="oh",
                                          tag="oh")
                          nc.vector.tensor_tensor(
                              oh[:, :nchi, :],
                              dlt[:, :nchi].to_broadcast([128, nchi, 128]),
                              iota4[:, 0:1, :].to_broadcast([128, nchi, 128]),
                              op=AL.is_equal)
                          for q in range(nchi):
                              t_, st, sp = cmeta[ch0 + q]
                              nc.tensor.matmul(
                                  eps_t[t_][:, :], lhsT=oh[:, q, :],
                                  rhs=exm[:, q, :], start=st, stop=sp)

                      # --- node phase per tile; MLP over the group ---
                      hog = None
                      h2qT = None
                      for ti, g in enumerate(grp):
                          tq = ti % 4
                          if tq == 0:
                              gs = min(4, ngt - ti)
                              hog = npool.tile([128, 4, C], F32, name="hog",
                                               tag="hog")
                              h2qT = npool.tile([C, 512], F32, name="h2qT",
                                                tag="h2qT")
                          nc.sync.dma_start(hog[:, tq, :],
                                            h_in[g * 128:(g + 1) * 128, 0:C])
                          dinv = npool.tile([128, C], F32, name="dinv", tag="dinv")
                          nc.vector.tensor_scalar(dinv[:, :], eps_t[ti][:, C:C2],
                                                  1e-16, None, AL.max)
                          nc.vector.reciprocal(dinv[:, :], dinv[:, :])
                          h2 = npool.tile([128, C], F32, name="h2", tag="h2")
                          nc.vector.tensor_tensor(h2[:, :], eps_t[ti][:, 0:C],
                                                  dinv[:, :], op=AL.mult)
                          nc.vector.tensor_tensor(h2[:, :], h2[:, :],
                                                  hog[:, tq, :], op=AL.add)
                          emit_q(nc, h2[:, :])
                          trp = ps_tr.tile([C, 128], F32, name="trp", tag="tr")
                          nc.tensor.transpose(trp[:, :], h2[:, :],
                                              identity=ident[:, :])
                          nc.vector.tensor_copy(h2qT[:, tq * 128:(tq + 1) * 128],
                                                trp[:, :])

                          if tq == gs - 1:
                              w = gs * 128
                              g0 = g - gs + 1
                              z1p = ps_z1.tile([C2, 512], F32, name="z1p", tag="z1p")
                              nc.tensor.matmul(z1p[:, :w], lhsT=W1q[l][:, :],
                                               rhs=h2qT[:, :w], start=True, stop=True)
                              z1s = npool.tile([C2, 512], F32, name="z1s", tag="z1s")
                              nc.scalar.activation(z1s[:, :w], z1p[:, :w],
                                                   ACTF.Identity,
                                                   bias=bias1[l][:, :], scale=QS)
                              nc.scalar.activation(z1s[:, :w], z1s[:, :w], ACTF.Copy,
                                                   bias=QB2, scale=QI)
                              nc.vector.tensor_scalar(z1s[:, :w], z1s[:, :w],
                                                      QMAX, QMIN, AL.min, AL.max)
                              nc.scalar.activation(z1s[:, :w], z1s[:, :w], ACTF.Relu,
                                                   bias=bi1[l][:, :], scale=sc1[l][:, :])
                              nc.scalar.activation(z1s[:, :w], z1s[:, :w], ACTF.Copy,
                                                   bias=MAGIC, scale=QS)
                              nc.scalar.activation(z1s[:, :w], z1s[:, :w], ACTF.Copy,
                                                   bias=QB2, scale=QI)
                              nc.vector.tensor_scalar(z1s[:, :w], z1s[:, :w],
                                                      QMAX, QMIN, AL.min, AL.max)
                              z2p = ps_z2.tile([C, 512], F32, name="z2p", tag="z2p")
                              nc.tensor.matmul(z2p[:, :w], lhsT=W2q[l][:, :],
                                               rhs=z1s[:, :w], start=True, stop=True)
                              z2s = npool.tile([C, 512], F32, name="z2s", tag="z2s")
                              nc.scalar.activation(z2s[:, :w], z2p[:, :w],
                                                   ACTF.Identity,
                                                   bias=bias2[l][:, :], scale=QS)
                              nc.scalar.activation(z2s[:, :w], z2s[:, :w], ACTF.Copy,
                                                   bias=QB2, scale=QI)
                              nc.vector.tensor_scalar(z2s[:, :w], z2s[:, :w],
                                                      QMAX, QMIN, AL.min, AL.max)
                              hnext = npool.tile([128, 4, C], F32, name="hnext",
                                                 tag="hnext")
                              for q in range(gs):
                                  trq = ps_tr.tile([128, C], F32, name="trq",
                                                   tag="tr")
                                  nc.tensor.transpose(trq[:, :],
                                                      z2s[:, q * 128:(q + 1) * 128],
                                                      identity=ident[0:C, 0:C])
                                  nc.vector.tensor_tensor(hnext[:, q, :], trq[:, :],
                                                          hog[:, q, :], op=AL.add)
                                  if last:
                                      blt = npool.tile([128, 1], F32, name="blt",
                                                       tag="blt")
                                      nc.sync.dma_start(
                                          blt[:, :],
                                          d_bloc[(g0 + q) * 128:(g0 + q + 1) * 128]
                                          .rearrange("(p one) -> p one", one=1))
                                      ohp = npool.tile([128, NW, 128], F32,
                                                       name="ohp", tag="ohp")
                                      nc.vector.tensor_tensor(
                                          ohp[:, :, :],
                                          blt[:, :].to_broadcast([128, NW, 128]),
                                          iota5[:, :, :], op=AL.is_equal)
                                      for wi in range(NW):
                                          pps = ps_pool.tile([128, C], F32,
                                                             name="pps", tag="pps")
                                          nc.tensor.matmul(
                                              pps[:, :], lhsT=ohp[:, wi, :],
                                              rhs=hnext[:, q, :],
                                              start=True, stop=True)
                                          nc.vector.tensor_tensor(
                                              pacc[:, wi, :], pacc[:, wi, :],
                                              pps[:, :], op=AL.add)
                              if not last:
                                  nc.sync.dma_start(
                                      h_out[g0 * 128:(g0 + gs) * 128, 0:C]
                                      .rearrange("(t p) c -> p t c", p=128),
                                      hnext[:, :gs, :])

                  if not last:
                      if cfg.use_collectives:
                          nc.gpsimd.collective_compute(
                              "AllGather", AL.bypass, replica_groups=RG,
                              ins=[h_out[:, :]], outs=[h64f[l + 1][:, :]])
                      else:
                          for b_ in range(cfg.ncores):
                              nc.sync.dma_start(
                                  h64f[l + 1][b_ * NB:(b_ + 1) * NB, :],
                                  h_out[:, :])

              if cfg.n_layers < L:
                  nl = cfg.n_layers
                  hf = h64f[min(nl, L - 1)]
                  for b_ in range(NPAD // 128):
                      dbg_t = npool.tile([128, C], F32, name="dbg_t", tag="dbg_t")
                      nc.sync.dma_start(dbg_t[:, :],
                                        hf[b_ * 128:(b_ + 1) * 128, 0:C])
                      nc.sync.dma_start(d_hdbg[b_ * 128:(b_ + 1) * 128, :],
                                        dbg_t[:, :])
                  return nc

              # ---- pooling: write window partials, AllReduce, output head ----
              nc.sync.dma_start(
                  pool_glob[:, :].rearrange("(w p) c -> p w c", p=128),
                  pacc[:, :, :])
              if cfg.use_collectives:
                  nc.gpsimd.collective_compute(
                      "AllReduce", AL.add, replica_groups=RG,
                      ins=[pool_glob[:, :]], outs=[pool_red[:, :]])
              else:
                  nc.sync.dma_start(pool_red[:, :], pool_glob[:, :])

              n_out_tiles = (G + 127) // 128
              for i in range(n_out_tiles):
                  w = min(128, G - i * 128)
                  pt = npool.tile([128, C], F32, name="pt", tag="pt")
                  nc.sync.dma_start(pt[:w, :], pool_red[i * 128:i * 128 + w, :])
                  civ = npool.tile([128, 1], F32, name="civ", tag="civ")
                  nc.sync.dma_start(civ[:w, :],
                                    d_cntinv[i * 128:i * 128 + w].rearrange("(p one) -> p one", one=1))
                  nc.vector.tensor_scalar(pt[:w, :], pt[:w, :], civ[:w, :], None, AL.mult)
                  emit_q(nc, pt[:w, :])
                  trh = ps_tr.tile([C, 128], F32, name="trh", tag="tr")
                  nc.tensor.transpose(trh[:, :w], pt[:w, :], identity=ident[:w, :w])
                  hts = npool.tile([C, 128], F32, name="hts", tag="hts")
                  nc.vector.tensor_copy(hts[:, :w], trh[:, :w])
                  op_ = ps_z2.tile([1, 128], F32, name="op_", tag="z2p")
                  nc.tensor.matmul(op_[:, :w], lhsT=Woq[:, :], rhs=hts[:, :w],
                                   start=True, stop=True)
                  osb = npool.tile([1, 128], F32, name="osb", tag="osb")
                  nc.scalar.activation(osb[:, :w], op_[:, :w], ACTF.Identity,
                                       bias=biaso[:, :], scale=QS)
                  nc.scalar.activation(osb[:, :w], osb[:, :w], ACTF.Copy,
                                       bias=QB2, scale=QI)
                  nc.vector.tensor_scalar(osb[:, :w], osb[:, :w], QMAX, QMIN,
                                          AL.min, AL.max)
                  nc.scalar.activation(osb[:, :w], osb[:, :w], ACTF.Sigmoid)
                  nc.scalar.activation(osb[:, :w], osb[:, :w], ACTF.Copy,
                                       bias=MAGIC, scale=QS)
                  nc.scalar.activation(osb[:, :w], osb[:, :w], ACTF.Copy,
                                       bias=QB2, scale=QI)
                  nc.sync.dma_start(
                      d_out[i * 128:i * 128 + w, :].rearrange("w one -> one w"),
                      osb[:, :w])

    return nc


# ----------------------------------------------------------------------------
# Entry point.
# ----------------------------------------------------------------------------

def run(inputs, cfg, **run_kwargs):
    global LAST_RESULTS
    in_maps = preprocess(inputs, cfg)
    nc = build(cfg)
    if not nc.is_finalized():
        nc.finalize()
    res = run_bass_kernel_spmd(nc, in_maps, core_ids=list(range(cfg.ncores)),
                               **run_kwargs)
    LAST_RESULTS = res
    return res.results[0]["out"].reshape(cfg.G, 1).astype(np.float32)


def kernel(**inputs) -> np.ndarray:
    cfg = Cfg(N=100000, E=3200000, G=512, XD=8, ED=4, C=32, L=4)
    return run(inputs, cfg)
